# revision 23
# baseline (speedup 1.0000x reference)
"""KgAdapterCrossAttention kernel for 8 trn2 NeuronCores.

Sharding: core = (batch b, query-half qh).  Each core computes attention for
1024 queries of one batch element against all 2048 keys.  221us -> 81us vs
the fp32 baseline (2.7x), all numerics within 1e-2 of the fp32 reference.

Design notes:
  - All matmuls use float32r / bf16 operands: 1 cycle/row on the PE instead
    of fp32's 4 (f32r keeps fp32 accuracy; inputs arrive as f32r via DMA,
    PSUM->SBUF copies act as the required f32r rounding ops).
  - Scores are computed pre-scaled by log2e*128 (folded into Wq on the host).
  - The 16 k-tiles of each (query-block, head) are processed in PAIRS that
    share a [128,1024] PSUM tile (two banks, sequential accumulation groups)
    so each exp instruction covers 1024 columns, amortizing the ~150ns
    access-latency overhead of Act/DVE instructions.  3 pair-slots rotate so
    both exp engines stream without stalling on PSUM.
  - exp splits across engines (pair schedule D,A,A,D,A,A,D,A): Act pairs use
    native Exp (scale=1/(log2e*128)) followed by a gpsimd align-mask
    multiply; DVE pairs use a fused fast-exp: ONE tensor_add of the PSUM
    scores with amq = mask ? 16250 : 8192 (i16), truncated to i16, whose
    bf16 BITCAST equals exp2(s*log2e) with the mask folded in (masked lanes
    land at ~2^-63).  16250 rather than 16256 centers the exponent-trick's
    linear-interpolation error (+-3%, zero mean, cancels in softmax).
  - P*V chains run per (qt, head); qt0 interleaves one head behind the score
    pipeline, qt1..3 defer into the NEXT query-block's head phases (pt tiles
    for heads 0/1 are double-buffered across blocks to allow the overlap).
    The softmax denominator rides along as a ones-column in V; normalize
    folds into the PSUM->SBUF copy as a per-partition scalar multiply.
  - O-projection transposes write into spare space of the O PSUM bank; the
    tail's qt2/3 accumulators borrow idle score-pool slots.
  - Mask DMAs are chunked per consumption unit and interleaved with the
    activation loads so arrivals pace the pipeline start.
"""

import os
import sys

import numpy as np
import ml_dtypes

try:
    import concourse.bass as bass
except ImportError:
    for _p in ("/opt/trn_rl_repo", os.path.expanduser("~/.axon_site/_ro/trn_rl_repo")):
        if os.path.isdir(_p) and _p not in sys.path:
            sys.path.insert(0, _p)
    import concourse.bass as bass

import concourse.mybir as mybir
import concourse.tile as tile
from concourse import bacc
from concourse.masks import make_identity
from contextlib import ExitStack

F32 = mybir.dt.float32
F32R = mybir.dt.float32r
BF16 = mybir.dt.bfloat16
I16 = mybir.dt.int16
EXP = mybir.ActivationFunctionType.Exp
ALU = mybir.AluOpType

P = 128
HID = 256
NHEAD = 4
DHEAD = 64
NQ = 1024  # queries per core
NK = 2048  # keys (full)
QBLK = 512
NQB = NQ // QBLK  # 2
NKT = NK // P  # 16
NPAIR = NKT // 2  # 8
NCT = HID // P  # 2

FE_MUL = float(np.log2(np.e)) * 128.0  # folded into Wq on host
ACT_SCALE = 1.0 / FE_MUL
AMQ_KEEP = 16250  # 127*128 minus centering delta 6
AMQ_KILL = 8192   # masked lanes -> bf16 2^-63 ~ 0

# engine per kt-pair: D = DVE fast-exp, A = Act exp (spread to avoid
# consecutive-A runs starving the 2-slot score-PSUM rotation)
PAIR_ENG = ["D", "A", "A", "D", "A", "A", "D", "A"]
ACT_KTS = [2, 3, 4, 5, 8, 9, 10, 11, 14, 15]
DVE_KTS = [0, 1, 6, 7, 12, 13]
ACT_POS = {kt: j for j, kt in enumerate(ACT_KTS)}
DVE_POS = {kt: j for j, kt in enumerate(DVE_KTS)}


def build() -> bass.Bass:
    nc = bacc.Bacc()
    xqT = nc.declare_dram_parameter("xqT", [HID, NQ], F32R, isOutput=False)
    xkT = nc.declare_dram_parameter("xkT", [HID, NK], F32R, isOutput=False)
    amf = nc.declare_dram_parameter("amf", [len(ACT_KTS) * P, NQ], BF16, isOutput=False)
    amq = nc.declare_dram_parameter("amq", [len(DVE_KTS) * P, NQ], I16, isOutput=False)
    wqT = nc.declare_dram_parameter("wqT", [HID, HID], F32R, isOutput=False)
    wkT = nc.declare_dram_parameter("wkT", [HID, HID], F32R, isOutput=False)
    wvT = nc.declare_dram_parameter("wvT", [HID, HID], F32R, isOutput=False)
    woT = nc.declare_dram_parameter("woT", [HID, HID], F32R, isOutput=False)
    out_d = nc.declare_dram_parameter("out", [NQ, HID], F32, isOutput=True)

    with tile.TileContext(nc) as tc, ExitStack() as ctx:
        const = ctx.enter_context(tc.tile_pool(name="const", bufs=1))
        big = ctx.enter_context(tc.tile_pool(name="big", bufs=1))
        ptp = ctx.enter_context(tc.tile_pool(name="ptp", bufs=1))
        amp = ctx.enter_context(tc.tile_pool(name="amp", bufs=1))
        wrk = ctx.enter_context(tc.tile_pool(name="wrk", bufs=2))
        ps_st = ctx.enter_context(tc.tile_pool(name="ps_st", bufs=3, space="PSUM"))
        ps_a = ctx.enter_context(tc.tile_pool(name="ps_a", bufs=1, space="PSUM"))
        ps_o = ctx.enter_context(tc.tile_pool(name="ps_o", bufs=1, space="PSUM"))

        # --- DMA loads (the DMA device serializes; K side first) ---
        def load2(name, src, width, dt=F32R):
            ts = []
            for t in range(2):
                tl = const.tile([P, width], dt, tag=f"{name}{t}", name=f"{name}{t}")
                nc.sync.dma_start(out=tl, in_=src[t * P : (t + 1) * P, :])
                ts.append(tl)
            return ts

        wk_sb = load2("wk", wkT, HID)
        xk_sb = []
        for t in range(2):
            tl = big.tile([P, NK], F32R, tag=f"xk{t}", name=f"xk{t}")
            nc.sync.dma_start(out=tl, in_=xkT[t * P : (t + 1) * P, :])
            xk_sb.append(tl)
        wq_sb = load2("wq", wqT, HID)
        wv_sb = load2("wv", wvT, HID)
        xq_sb = []
        for t in range(2):
            tl = big.tile([P, NQ], F32R, tag=f"xq{t}", name=f"xq{t}")
            nc.sync.dma_start(out=tl, in_=xqT[t * P : (t + 1) * P, :])
            xq_sb.append(tl)

        amf_r = amf.rearrange("(t p) q -> p t q", p=P)
        amq_r = amq.rearrange("(t p) q -> p t q", p=P)
        am_sb = {}   # (qb, act_pos_start) -> tile [P, w, QBLK] bf16
        amq_sb = {}  # (qb, dve_pos_start) -> tile [P, w, QBLK] i16

        def load_mask_chunks(qb):
            # one chunk per consumption unit, in pair order, so the DMA
            # arrivals pace the pipeline
            qsl = slice(qb * QBLK, (qb + 1) * QBLK)
            a_pos = 0
            d_pos = 0
            for pj in range(NPAIR):
                eng = PAIR_ENG[pj]
                if eng in ("D", "S"):
                    w = 1 if eng == "S" else 2
                    tq = amp.tile(
                        [P, w, QBLK], I16, tag=f"amq{qb}_{d_pos}", name=f"amq{qb}_{d_pos}"
                    )
                    nc.sync.dma_start(out=tq, in_=amq_r[:, d_pos : d_pos + w, qsl])
                    amq_sb[(qb, d_pos)] = tq
                    d_pos += w
                if eng in ("A", "S"):
                    w = 1 if eng == "S" else 2
                    tl = amp.tile(
                        [P, w, QBLK], BF16, tag=f"am{qb}_{a_pos}", name=f"am{qb}_{a_pos}"
                    )
                    nc.sync.dma_start(out=tl, in_=amf_r[:, a_pos : a_pos + w, qsl])
                    am_sb[(qb, a_pos)] = tl
                    a_pos += w

        load_mask_chunks(0)
        wo_sb = load2("wo", woT, HID)
        load_mask_chunks(1)

        ident_f = const.tile([P, P], F32, tag="ident_f", name="ident_f")
        make_identity(nc, ident_f)
        ident = const.tile([P, P], F32R, tag="ident", name="ident")
        nc.vector.tensor_copy(ident, ident_f)

        # --- projections (copies alternate Act/DVE; prologue work) ---
        cp_i = 0

        def copy_eng(out, in_):
            nonlocal cp_i
            cp_i += 1
            if cp_i % 2 == 0:
                nc.scalar.copy(out, in_)
            else:
                nc.vector.tensor_copy(out, in_)

        kt_sb = [big.tile([P, NK], F32R, tag=f"kt{t}", name=f"kt{t}") for t in range(2)]
        for t in range(2):
            for nb in range(NK // (2 * QBLK)):
                ps = ps_st.tile([P, 2 * QBLK], F32, tag="st", name="st")
                for half in range(2):
                    for ct in range(NCT):
                        nc.tensor.matmul(
                            ps[:, half * QBLK : (half + 1) * QBLK],
                            lhsT=wk_sb[ct][:, t * P : (t + 1) * P],
                            rhs=xk_sb[ct][
                                :, (2 * nb + half) * QBLK : (2 * nb + half + 1) * QBLK
                            ],
                            start=(ct == 0),
                            stop=(ct == NCT - 1),
                        )
                copy_eng(kt_sb[t][:, 2 * nb * QBLK : (2 * nb + 2) * QBLK], ps)

        qt_sb = [big.tile([P, NQ], F32R, tag=f"qt{t}", name=f"qt{t}") for t in range(2)]
        for t in range(2):
            ps = ps_st.tile([P, 2 * QBLK], F32, tag="st", name="st")
            for half in range(2):
                for ct in range(NCT):
                    nc.tensor.matmul(
                        ps[:, half * QBLK : (half + 1) * QBLK],
                        lhsT=wq_sb[ct][:, t * P : (t + 1) * P],
                        rhs=xq_sb[ct][:, half * QBLK : (half + 1) * QBLK],
                        start=(ct == 0),
                        stop=(ct == NCT - 1),
                    )
            copy_eng(qt_sb[t], ps)

        # V''[ktok, h, 0:64] = V rows (bf16); V''[ktok, h, 64] = 1.0
        vpp = [None] * NKT
        for kt2 in range(NKT // 2):
            ps = ps_st.tile([P, 2 * QBLK], F32, tag="st", name="st")
            for half in range(2):
                kt = 2 * kt2 + half
                for ct in range(NCT):
                    nc.tensor.matmul(
                        ps[:, half * QBLK : half * QBLK + HID],
                        lhsT=xk_sb[ct][:, kt * P : (kt + 1) * P],
                        rhs=wv_sb[ct],
                        start=(ct == 0),
                        stop=(ct == NCT - 1),
                    )
            for half in range(2):
                kt = 2 * kt2 + half
                tl = big.tile([P, NHEAD, DHEAD + 1], BF16, tag=f"v{kt}", name=f"v{kt}")
                copy_eng(
                    tl[:, :, 0:DHEAD],
                    ps[:, half * QBLK : half * QBLK + HID].rearrange(
                        "p (h d) -> p h d", h=NHEAD
                    ),
                )
                nc.gpsimd.memset(tl[:, :, DHEAD : DHEAD + 1], 1.0)
                vpp[kt] = tl

        # --- attention ---
        def emit_head(qb, h, pts):
            qsl = slice(qb * QBLK, (qb + 1) * QBLK)
            t, po = h // 2, (h % 2) * DHEAD
            for pj in range(NPAIR):
                ps = ps_st.tile([P, 2 * QBLK], F32, tag="st", name="st")
                for half in range(2):
                    kt = 2 * pj + half
                    nc.tensor.matmul(
                        ps[:, half * QBLK : (half + 1) * QBLK],
                        lhsT=kt_sb[t][po : po + DHEAD, kt * P : (kt + 1) * P],
                        rhs=qt_sb[t][po : po + DHEAD, qsl],
                        start=True,
                        stop=True,
                    )
                eng = PAIR_ENG[pj]
                tg = f"pt{h}_{pj}" if h > 1 else f"pt{h}{qb % 2}_{pj}"
                if eng == "A":
                    pt = ptp.tile([P, 2 * QBLK], BF16, tag=tg, name=tg)
                    nc.scalar.activation(pt, ps, EXP, scale=ACT_SCALE)
                    j = ACT_POS[2 * pj]
                    nc.gpsimd.tensor_mul(
                        pt.rearrange("p (two q) -> p two q", two=2),
                        pt.rearrange("p (two q) -> p two q", two=2),
                        am_sb[(qb, j)],
                    )
                    for half in range(2):
                        pts[(h, 2 * pj + half)] = pt[:, half * QBLK : (half + 1) * QBLK]
                elif eng == "D":
                    ie = ptp.tile([P, 2 * QBLK], I16, tag=tg, name=tg)
                    j = DVE_POS[2 * pj]
                    nc.vector.tensor_add(
                        ie.rearrange("p (two q) -> p two q", two=2),
                        ps.rearrange("p (two q) -> p two q", two=2),
                        amq_sb[(qb, j)],
                    )
                    pv = ie.bitcast(BF16)
                    for half in range(2):
                        pts[(h, 2 * pj + half)] = pv[:, half * QBLK : (half + 1) * QBLK]
                else:  # split pair: kt14 -> DVE, kt15 -> Act
                    ie = ptp.tile([P, QBLK], I16, tag=f"pti{h}_{pj}", name=f"pti{h}_{pj}")
                    jd = DVE_POS[2 * pj]
                    nc.vector.tensor_add(
                        ie, ps[:, 0:QBLK], amq_sb[(qb, jd)][:, 0, :]
                    )
                    pts[(h, 2 * pj)] = ie.bitcast(BF16)
                    pt = ptp.tile([P, QBLK], BF16, tag=f"pta{h}_{pj}", name=f"pta{h}_{pj}")
                    nc.scalar.activation(pt, ps[:, QBLK : 2 * QBLK], EXP, scale=ACT_SCALE)
                    ja = ACT_POS[2 * pj + 1]
                    nc.gpsimd.tensor_mul(pt, pt, am_sb[(qb, ja)][:, 0, :])
                    pts[(h, 2 * pj + 1)] = pt

        deferred = [None]

        def run_deferred():
            if deferred[0] is not None:
                deferred[0]()
                deferred[0] = None

        def process_qb(qb):
            pts = {}
            ps_avs = {}

            def get_ps_av(qt):
                if qt not in ps_avs:
                    if qt < 2:
                        ps_avs[qt] = ps_a.tile(
                            [P, NHEAD * (DHEAD + 1)], F32, tag="a0",
                            name="a0", padded_shape=[P, QBLK],
                        )
                    else:
                        # tail-only: borrow an idle score-pool slot
                        big_t = ps_st.tile([P, 2 * QBLK], F32, tag="st", name="st_a")
                        ps_avs[qt] = big_t[:, 0 : NHEAD * (DHEAD + 1)]
                return ps_avs[qt]

            def emit_chain(qt, h):
                ps_av = get_ps_av(qt)
                for kt in range(NKT):
                    nc.tensor.matmul(
                        ps_av[:, h * 65 : (h + 1) * 65],
                        lhsT=pts[(h, kt)][:, qt * P : (qt + 1) * P],
                        rhs=vpp[kt][:, h, :],
                        start=(kt == 0),
                        stop=(kt == NKT - 1),
                    )

            def emit_norm(qt):
                ps_av = ps_avs.pop(qt)
                rec = wrk.tile([P, NHEAD], F32, tag=f"rec{qt % 2}", name=f"rec{qt % 2}")
                nc.vector.reciprocal(rec, ps_av[:, DHEAD : NHEAD * 65 : 65])
                an = wrk.tile([P, HID], F32R, tag=f"an{qt % 2}", name=f"an{qt % 2}")
                for h in range(NHEAD):
                    nc.vector.tensor_scalar_mul(
                        an[:, h * DHEAD : (h + 1) * DHEAD],
                        ps_av[:, h * 65 : h * 65 + DHEAD],
                        rec[:, h : h + 1],
                    )
                return an

            def emit_o(qt, an):
                o_ps = ps_o.tile([P, QBLK], F32, tag="o", name="o_ps")
                atts = []
                for ct in range(NCT):
                    tp = o_ps[:, HID + ct * P : HID + (ct + 1) * P].bitcast(F32R)
                    nc.tensor.transpose(tp, an[:, ct * P : (ct + 1) * P], ident)
                    att = wrk.tile([P, P], F32R, tag=f"att{ct}", name=f"att{ct}")
                    nc.vector.tensor_copy(att, tp)
                    atts.append(att)
                for ct in range(NCT):
                    nc.tensor.matmul(
                        o_ps[:, 0:HID],
                        lhsT=atts[ct],
                        rhs=wo_sb[ct],
                        start=(ct == 0),
                        stop=(ct == NCT - 1),
                    )
                ob = wrk.tile([P, HID], F32, tag=f"ob{qt % 2}", name=f"ob{qt % 2}")
                if qt % 2 == 0:
                    nc.scalar.copy(ob, o_ps[:, 0:HID])
                else:
                    nc.vector.tensor_copy(ob, o_ps[:, 0:HID])
                q0 = qb * QBLK + qt * P
                nc.sync.dma_start(out=out_d[q0 : q0 + P, :], in_=ob)

            for h in range(NHEAD):
                emit_head(qb, h, pts)
                if h == 1:
                    # previous qb's deferred tail chains/outputs overlap
                    # this qb's first head phases
                    run_deferred()
                if h > 0:
                    emit_chain(0, h - 1)
            emit_chain(0, NHEAD - 1)
            an0 = emit_norm(0)

            def tail(emit_chain=emit_chain, emit_norm=emit_norm,
                     emit_o=emit_o, an0=an0):
                emit_o(0, an0)
                for h in range(NHEAD):
                    for qt in range(1, 4):
                        emit_chain(qt, h)
                an1 = emit_norm(1)
                emit_o(1, an1)
                an2 = emit_norm(2)
                emit_o(2, an2)
                an3 = emit_norm(3)
                emit_o(3, an3)

            deferred[0] = tail

        for qb in range(NQB):
            process_qb(qb)
        run_deferred()
    nc.compile()
    return nc


_NC_CACHE = {}
_last_in_maps = None


def _get_nc(with_attn_mask: bool = False) -> bass.Bass:
    key = "v5"
    if key not in _NC_CACHE:
        _NC_CACHE[key] = build()
    return _NC_CACHE[key]


def kernel(q_hidden_states, k_hidden_states, attention_mask, align_mask, Wq, Wk, Wv, Wo):
    from concourse.bass_utils import run_bass_kernel_spmd

    q_hidden_states = np.asarray(q_hidden_states, np.float32)
    k_hidden_states = np.asarray(k_hidden_states, np.float32)
    attention_mask = np.asarray(attention_mask, np.float32)
    align_mask = np.asarray(align_mask)
    B, Q, _ = q_hidden_states.shape
    qh_len = Q // 2  # 1024

    nc = _get_nc()

    # scores arrive in PSUM pre-scaled by log2e*128 (folded into Wq here)
    wq = np.ascontiguousarray(np.asarray(Wq, np.float32).T) * np.float32(FE_MUL / 8.0)
    wk = np.ascontiguousarray(np.asarray(Wk, np.float32).T)
    wv = np.ascontiguousarray(np.asarray(Wv, np.float32).T)
    wo = np.ascontiguousarray(np.asarray(Wo, np.float32).T)

    if np.any(attention_mask):
        raise NotImplementedError("nonzero additive attention_mask not supported")

    in_maps = []
    for core in range(8):
        b, qh = divmod(core, 2)
        qsl = slice(qh * qh_len, (qh + 1) * qh_len)
        am = align_mask[b, :, qsl]
        am_act = np.concatenate([am[kt * 128 : (kt + 1) * 128] for kt in ACT_KTS])
        am_dve = np.concatenate([am[kt * 128 : (kt + 1) * 128] for kt in DVE_KTS])
        m = {
            "xqT": np.ascontiguousarray(q_hidden_states[b, qsl].T),
            "xkT": np.ascontiguousarray(k_hidden_states[b].T),
            "amf": np.ascontiguousarray(am_act.astype(ml_dtypes.bfloat16)),
            "amq": np.ascontiguousarray(
                np.where(am_dve != 0, AMQ_KEEP, AMQ_KILL).astype(np.int16)
            ),
            "wqT": wq,
            "wkT": wk,
            "wvT": wv,
            "woT": wo,
        }
        in_maps.append(m)

    global _last_in_maps
    _last_in_maps = in_maps
    res = run_bass_kernel_spmd(nc, in_maps, list(range(8))).results
    out = np.empty((B, Q, HID), np.float32)
    for core in range(8):
        b, qh = divmod(core, 2)
        out[b, qh * qh_len : (qh + 1) * qh_len] = res[core]["out"]
    return out


# revision 25
# speedup vs baseline: 1.0062x; 1.0062x over previous
"""KgAdapterCrossAttention kernel for 8 trn2 NeuronCores.

Sharding: core = (batch b, query-half qh).  Each core computes attention for
1024 queries of one batch element against all 2048 keys.  221us -> 81us vs
the fp32 baseline (2.7x), all numerics within 1e-2 of the fp32 reference.

Design notes:
  - All matmuls use float32r / bf16 operands: 1 cycle/row on the PE instead
    of fp32's 4 (f32r keeps fp32 accuracy; inputs arrive as f32r via DMA,
    PSUM->SBUF copies act as the required f32r rounding ops).
  - Scores are computed pre-scaled by log2e*128 (folded into Wq on the host).
  - The 16 k-tiles of each (query-block, head) are processed in PAIRS that
    share a [128,1024] PSUM tile (two banks, sequential accumulation groups)
    so each exp instruction covers 1024 columns, amortizing the ~150ns
    access-latency overhead of Act/DVE instructions.  3 pair-slots rotate so
    both exp engines stream without stalling on PSUM.
  - exp splits across engines (pair schedule D,A,A,D,A,A,D,A): Act pairs use
    native Exp (scale=1/(log2e*128)) followed by a gpsimd align-mask
    multiply; DVE pairs use a fused fast-exp: ONE tensor_add of the PSUM
    scores with amq = mask ? 16250 : 8192 (i16), truncated to i16, whose
    bf16 BITCAST equals exp2(s*log2e) with the mask folded in (masked lanes
    land at ~2^-63).  16250 rather than 16256 centers the exponent-trick's
    linear-interpolation error (+-3%, zero mean, cancels in softmax).
  - P*V chains run per (qt, head); qt0 interleaves one head behind the score
    pipeline, qt1..3 defer into the NEXT query-block's head phases (pt tiles
    for heads 0/1 are double-buffered across blocks to allow the overlap).
    The softmax denominator rides along as a ones-column in V; normalize
    folds into the PSUM->SBUF copy as a per-partition scalar multiply.
  - O-projection transposes write into spare space of the O PSUM bank; the
    tail's qt2/3 accumulators borrow idle score-pool slots.
  - Mask DMAs are chunked per consumption unit and interleaved with the
    activation loads so arrivals pace the pipeline start.
"""

import os
import sys

import numpy as np
import ml_dtypes

try:
    import concourse.bass as bass
except ImportError:
    for _p in ("/opt/trn_rl_repo", os.path.expanduser("~/.axon_site/_ro/trn_rl_repo")):
        if os.path.isdir(_p) and _p not in sys.path:
            sys.path.insert(0, _p)
    import concourse.bass as bass

import concourse.mybir as mybir
import concourse.tile as tile
from concourse import bacc
from concourse.masks import make_identity
from contextlib import ExitStack

F32 = mybir.dt.float32
F32R = mybir.dt.float32r
BF16 = mybir.dt.bfloat16
I16 = mybir.dt.int16
EXP = mybir.ActivationFunctionType.Exp
ALU = mybir.AluOpType

P = 128
HID = 256
NHEAD = 4
DHEAD = 64
NQ = 1024  # queries per core
NK = 2048  # keys (full)
QBLK = 512
NQB = NQ // QBLK  # 2
NKT = NK // P  # 16
NPAIR = NKT // 2  # 8
NCT = HID // P  # 2

FE_MUL = float(np.log2(np.e)) * 128.0  # folded into Wq on host
ACT_SCALE = 1.0 / FE_MUL
AMQ_KEEP = 16250  # 127*128 minus centering delta 6
AMQ_KILL = 8192   # masked lanes -> bf16 2^-63 ~ 0

# engine per kt-pair: D = DVE fast-exp, A = Act exp (spread to avoid
# consecutive-A runs starving the 2-slot score-PSUM rotation)
PAIR_ENG = ["D", "A", "A", "D", "A", "A", "D", "A"]
ACT_KTS = [2, 3, 4, 5, 8, 9, 10, 11, 14, 15]
DVE_KTS = [0, 1, 6, 7, 12, 13]
ACT_POS = {kt: j for j, kt in enumerate(ACT_KTS)}
DVE_POS = {kt: j for j, kt in enumerate(DVE_KTS)}


def build() -> bass.Bass:
    nc = bacc.Bacc()
    xqT = nc.declare_dram_parameter("xqT", [HID, NQ], F32R, isOutput=False)
    xkT = nc.declare_dram_parameter("xkT", [HID, NK], F32R, isOutput=False)
    amf = nc.declare_dram_parameter("amf", [len(ACT_KTS) * P, NQ], BF16, isOutput=False)
    amq = nc.declare_dram_parameter("amq", [len(DVE_KTS) * P, NQ], I16, isOutput=False)
    wqT = nc.declare_dram_parameter("wqT", [HID, HID], F32R, isOutput=False)
    wkT = nc.declare_dram_parameter("wkT", [HID, HID], F32R, isOutput=False)
    wvT = nc.declare_dram_parameter("wvT", [HID, HID], F32R, isOutput=False)
    woT = nc.declare_dram_parameter("woT", [HID, HID], F32R, isOutput=False)
    out_d = nc.declare_dram_parameter("out", [NQ, HID], F32, isOutput=True)

    with tile.TileContext(nc) as tc, ExitStack() as ctx:
        const = ctx.enter_context(tc.tile_pool(name="const", bufs=1))
        big = ctx.enter_context(tc.tile_pool(name="big", bufs=1))
        ptp = ctx.enter_context(tc.tile_pool(name="ptp", bufs=1))
        amp = ctx.enter_context(tc.tile_pool(name="amp", bufs=1))
        wrk = ctx.enter_context(tc.tile_pool(name="wrk", bufs=2))
        wrk1 = ctx.enter_context(tc.tile_pool(name="wrk1", bufs=1))
        ps_st = ctx.enter_context(tc.tile_pool(name="ps_st", bufs=3, space="PSUM"))
        ps_a = ctx.enter_context(tc.tile_pool(name="ps_a", bufs=1, space="PSUM"))
        ps_o = ctx.enter_context(tc.tile_pool(name="ps_o", bufs=1, space="PSUM"))

        # --- DMA loads (the DMA device serializes; K side first) ---
        def load2(name, src, width, dt=F32R):
            ts = []
            for t in range(2):
                tl = const.tile([P, width], dt, tag=f"{name}{t}", name=f"{name}{t}")
                nc.sync.dma_start(out=tl, in_=src[t * P : (t + 1) * P, :])
                ts.append(tl)
            return ts

        wk_sb = load2("wk", wkT, HID)
        xk_sb = []
        for t in range(2):
            tl = big.tile([P, NK], F32R, tag=f"xk{t}", name=f"xk{t}")
            nc.sync.dma_start(out=tl, in_=xkT[t * P : (t + 1) * P, :])
            xk_sb.append(tl)
        wq_sb = load2("wq", wqT, HID)
        wv_sb = load2("wv", wvT, HID)
        xq_sb = []
        for t in range(2):
            tl = big.tile([P, NQ], F32R, tag=f"xq{t}", name=f"xq{t}")
            nc.sync.dma_start(out=tl, in_=xqT[t * P : (t + 1) * P, :])
            xq_sb.append(tl)

        amf_r = amf.rearrange("(t p) q -> p t q", p=P)
        amq_r = amq.rearrange("(t p) q -> p t q", p=P)
        am_sb = {}   # (qb, act_pos_start) -> tile [P, w, QBLK] bf16
        amq_sb = {}  # (qb, dve_pos_start) -> tile [P, w, QBLK] i16

        def load_mask_chunks(qb):
            # one chunk per consumption unit, in pair order, so the DMA
            # arrivals pace the pipeline
            qsl = slice(qb * QBLK, (qb + 1) * QBLK)
            a_pos = 0
            d_pos = 0
            for pj in range(NPAIR):
                eng = PAIR_ENG[pj]
                if eng in ("D", "S"):
                    w = 1 if eng == "S" else 2
                    tq = amp.tile(
                        [P, w, QBLK], I16, tag=f"amq_{d_pos}", name=f"amq_{d_pos}"
                    )
                    nc.sync.dma_start(out=tq, in_=amq_r[:, d_pos : d_pos + w, qsl])
                    amq_sb[(qb, d_pos)] = tq
                    d_pos += w
                if eng in ("A", "S"):
                    w = 1 if eng == "S" else 2
                    tl = amp.tile(
                        [P, w, QBLK], BF16, tag=f"am_{a_pos}", name=f"am_{a_pos}"
                    )
                    nc.sync.dma_start(out=tl, in_=amf_r[:, a_pos : a_pos + w, qsl])
                    am_sb[(qb, a_pos)] = tl
                    a_pos += w

        load_mask_chunks(0)
        wo_sb = load2("wo", woT, HID)

        ident_f = const.tile([P, P], F32, tag="ident_f", name="ident_f")
        make_identity(nc, ident_f)
        ident = const.tile([P, P], F32R, tag="ident", name="ident")
        nc.vector.tensor_copy(ident, ident_f)

        # --- projections (copies alternate Act/DVE; prologue work) ---
        cp_i = 0

        def copy_eng(out, in_):
            nonlocal cp_i
            cp_i += 1
            if cp_i % 2 == 0:
                nc.scalar.copy(out, in_)
            else:
                nc.vector.tensor_copy(out, in_)

        kt_sb = [big.tile([P, NK], F32R, tag=f"kt{t}", name=f"kt{t}") for t in range(2)]
        for t in range(2):
            for nb in range(NK // (2 * QBLK)):
                ps = ps_st.tile([P, 2 * QBLK], F32, tag="st", name="st")
                for half in range(2):
                    for ct in range(NCT):
                        nc.tensor.matmul(
                            ps[:, half * QBLK : (half + 1) * QBLK],
                            lhsT=wk_sb[ct][:, t * P : (t + 1) * P],
                            rhs=xk_sb[ct][
                                :, (2 * nb + half) * QBLK : (2 * nb + half + 1) * QBLK
                            ],
                            start=(ct == 0),
                            stop=(ct == NCT - 1),
                        )
                copy_eng(kt_sb[t][:, 2 * nb * QBLK : (2 * nb + 2) * QBLK], ps)

        qt_sb = [big.tile([P, NQ], F32R, tag=f"qt{t}", name=f"qt{t}") for t in range(2)]
        for t in range(2):
            ps = ps_st.tile([P, 2 * QBLK], F32, tag="st", name="st")
            for half in range(2):
                for ct in range(NCT):
                    nc.tensor.matmul(
                        ps[:, half * QBLK : (half + 1) * QBLK],
                        lhsT=wq_sb[ct][:, t * P : (t + 1) * P],
                        rhs=xq_sb[ct][:, half * QBLK : (half + 1) * QBLK],
                        start=(ct == 0),
                        stop=(ct == NCT - 1),
                    )
            copy_eng(qt_sb[t], ps)

        # V''[ktok, h, 0:64] = V rows (bf16); V''[ktok, h, 64] = 1.0
        vpp = [None] * NKT
        for kt2 in range(NKT // 2):
            ps = ps_st.tile([P, 2 * QBLK], F32, tag="st", name="st")
            for half in range(2):
                kt = 2 * kt2 + half
                for ct in range(NCT):
                    nc.tensor.matmul(
                        ps[:, half * QBLK : half * QBLK + HID],
                        lhsT=xk_sb[ct][:, kt * P : (kt + 1) * P],
                        rhs=wv_sb[ct],
                        start=(ct == 0),
                        stop=(ct == NCT - 1),
                    )
            for half in range(2):
                kt = 2 * kt2 + half
                tl = big.tile([P, NHEAD, DHEAD + 1], BF16, tag=f"v{kt}", name=f"v{kt}")
                copy_eng(
                    tl[:, :, 0:DHEAD],
                    ps[:, half * QBLK : half * QBLK + HID].rearrange(
                        "p (h d) -> p h d", h=NHEAD
                    ),
                )
                nc.gpsimd.memset(tl[:, :, DHEAD : DHEAD + 1], 1.0)
                vpp[kt] = tl

        # --- attention ---
        def emit_head(qb, h, pts):
            qsl = slice(qb * QBLK, (qb + 1) * QBLK)
            t, po = h // 2, (h % 2) * DHEAD
            for pj in range(NPAIR):
                ps = ps_st.tile([P, 2 * QBLK], F32, tag="st", name="st")
                for half in range(2):
                    kt = 2 * pj + half
                    nc.tensor.matmul(
                        ps[:, half * QBLK : (half + 1) * QBLK],
                        lhsT=kt_sb[t][po : po + DHEAD, kt * P : (kt + 1) * P],
                        rhs=qt_sb[t][po : po + DHEAD, qsl],
                        start=True,
                        stop=True,
                    )
                eng = PAIR_ENG[pj]
                tg = f"pt{h}_{pj}" if h > 2 else f"pt{h}{qb % 2}_{pj}"
                if eng == "A":
                    pt = ptp.tile([P, 2 * QBLK], BF16, tag=tg, name=tg)
                    nc.scalar.activation(pt, ps, EXP, scale=ACT_SCALE)
                    j = ACT_POS[2 * pj]
                    nc.gpsimd.tensor_mul(
                        pt.rearrange("p (two q) -> p two q", two=2),
                        pt.rearrange("p (two q) -> p two q", two=2),
                        am_sb[(qb, j)],
                    )
                    for half in range(2):
                        pts[(h, 2 * pj + half)] = pt[:, half * QBLK : (half + 1) * QBLK]
                elif eng == "D":
                    ie = ptp.tile([P, 2 * QBLK], I16, tag=tg, name=tg)
                    j = DVE_POS[2 * pj]
                    nc.vector.tensor_add(
                        ie.rearrange("p (two q) -> p two q", two=2),
                        ps.rearrange("p (two q) -> p two q", two=2),
                        amq_sb[(qb, j)],
                    )
                    pv = ie.bitcast(BF16)
                    for half in range(2):
                        pts[(h, 2 * pj + half)] = pv[:, half * QBLK : (half + 1) * QBLK]
                else:  # split pair: kt14 -> DVE, kt15 -> Act
                    ie = ptp.tile([P, QBLK], I16, tag=f"pti{h}_{pj}", name=f"pti{h}_{pj}")
                    jd = DVE_POS[2 * pj]
                    nc.vector.tensor_add(
                        ie, ps[:, 0:QBLK], amq_sb[(qb, jd)][:, 0, :]
                    )
                    pts[(h, 2 * pj)] = ie.bitcast(BF16)
                    pt = ptp.tile([P, QBLK], BF16, tag=f"pta{h}_{pj}", name=f"pta{h}_{pj}")
                    nc.scalar.activation(pt, ps[:, QBLK : 2 * QBLK], EXP, scale=ACT_SCALE)
                    ja = ACT_POS[2 * pj + 1]
                    nc.gpsimd.tensor_mul(pt, pt, am_sb[(qb, ja)][:, 0, :])
                    pts[(h, 2 * pj + 1)] = pt

        deferred = [None]

        def run_deferred():
            if deferred[0] is not None:
                deferred[0]()
                deferred[0] = None

        def process_qb(qb):
            if qb > 0:
                load_mask_chunks(qb)
            pts = {}
            ps_avs = {}

            def get_ps_av(qt):
                if qt not in ps_avs:
                    if qt < 2:
                        ps_avs[qt] = ps_a.tile(
                            [P, NHEAD * (DHEAD + 1)], F32, tag="a0",
                            name="a0", padded_shape=[P, QBLK],
                        )
                    else:
                        # tail-only: borrow an idle score-pool slot
                        big_t = ps_st.tile([P, 2 * QBLK], F32, tag="st", name="st_a")
                        ps_avs[qt] = big_t[:, 0 : NHEAD * (DHEAD + 1)]
                return ps_avs[qt]

            def emit_chain(qt, h):
                ps_av = get_ps_av(qt)
                for kt in range(NKT):
                    nc.tensor.matmul(
                        ps_av[:, h * 65 : (h + 1) * 65],
                        lhsT=pts[(h, kt)][:, qt * P : (qt + 1) * P],
                        rhs=vpp[kt][:, h, :],
                        start=(kt == 0),
                        stop=(kt == NKT - 1),
                    )

            def emit_norm(qt):
                ps_av = ps_avs.pop(qt)
                rec = wrk.tile([P, NHEAD], F32, tag=f"rec{qt % 2}", name=f"rec{qt % 2}")
                nc.vector.reciprocal(rec, ps_av[:, DHEAD : NHEAD * 65 : 65])
                an = wrk1.tile([P, HID], F32R, tag=f"an{qt % 2}", name=f"an{qt % 2}")
                for h in range(NHEAD):
                    nc.vector.tensor_scalar_mul(
                        an[:, h * DHEAD : (h + 1) * DHEAD],
                        ps_av[:, h * 65 : h * 65 + DHEAD],
                        rec[:, h : h + 1],
                    )
                return an

            def emit_o(qt, an):
                o_ps = ps_o.tile([P, QBLK], F32, tag="o", name="o_ps")
                atts = []
                for ct in range(NCT):
                    tp = o_ps[:, HID + ct * P : HID + (ct + 1) * P].bitcast(F32R)
                    nc.tensor.transpose(tp, an[:, ct * P : (ct + 1) * P], ident)
                    att = wrk.tile([P, P], F32R, tag=f"att{ct}", name=f"att{ct}")
                    nc.vector.tensor_copy(att, tp)
                    atts.append(att)
                for ct in range(NCT):
                    nc.tensor.matmul(
                        o_ps[:, 0:HID],
                        lhsT=atts[ct],
                        rhs=wo_sb[ct],
                        start=(ct == 0),
                        stop=(ct == NCT - 1),
                    )
                ob = wrk1.tile([P, HID], F32, tag=f"ob{qt % 2}", name=f"ob{qt % 2}")
                if qt % 2 == 0:
                    nc.scalar.copy(ob, o_ps[:, 0:HID])
                else:
                    nc.vector.tensor_copy(ob, o_ps[:, 0:HID])
                q0 = qb * QBLK + qt * P
                nc.sync.dma_start(out=out_d[q0 : q0 + P, :], in_=ob)

            for h in range(NHEAD):
                emit_head(qb, h, pts)
                if h == 1:
                    # previous qb's deferred tail chains/outputs overlap
                    # this qb's first head phases
                    run_deferred()
                if h > 0:
                    emit_chain(0, h - 1)
            emit_chain(0, NHEAD - 1)
            an0 = emit_norm(0)

            last = qb == NQB - 1

            def tail(emit_chain=emit_chain, emit_norm=emit_norm,
                     emit_o=emit_o, an0=an0, last=last):
                emit_o(0, an0)
                if last:
                    # terminal: qt-major so norms/outputs pipeline with the
                    # following qt's chains
                    for qt in range(1, 4):
                        for h in range(NHEAD):
                            emit_chain(qt, h)
                        an = emit_norm(qt)
                        emit_o(qt, an)
                else:
                    # overlapped with the next block: h-major frees pt tiles
                    # for the next block's exps as early as possible
                    for h in range(NHEAD):
                        for qt in range(1, 4):
                            emit_chain(qt, h)
                    an1 = emit_norm(1)
                    emit_o(1, an1)
                    an2 = emit_norm(2)
                    emit_o(2, an2)
                    an3 = emit_norm(3)
                    emit_o(3, an3)

            deferred[0] = tail

        for qb in range(NQB):
            process_qb(qb)
        run_deferred()
    nc.compile()
    return nc


_NC_CACHE = {}
_last_in_maps = None


def _get_nc(with_attn_mask: bool = False) -> bass.Bass:
    key = "v5"
    if key not in _NC_CACHE:
        _NC_CACHE[key] = build()
    return _NC_CACHE[key]


def kernel(q_hidden_states, k_hidden_states, attention_mask, align_mask, Wq, Wk, Wv, Wo):
    from concourse.bass_utils import run_bass_kernel_spmd

    q_hidden_states = np.asarray(q_hidden_states, np.float32)
    k_hidden_states = np.asarray(k_hidden_states, np.float32)
    attention_mask = np.asarray(attention_mask, np.float32)
    align_mask = np.asarray(align_mask)
    B, Q, _ = q_hidden_states.shape
    qh_len = Q // 2  # 1024

    nc = _get_nc()

    # scores arrive in PSUM pre-scaled by log2e*128 (folded into Wq here)
    wq = np.ascontiguousarray(np.asarray(Wq, np.float32).T) * np.float32(FE_MUL / 8.0)
    wk = np.ascontiguousarray(np.asarray(Wk, np.float32).T)
    wv = np.ascontiguousarray(np.asarray(Wv, np.float32).T)
    wo = np.ascontiguousarray(np.asarray(Wo, np.float32).T)

    if np.any(attention_mask):
        raise NotImplementedError("nonzero additive attention_mask not supported")

    in_maps = []
    for core in range(8):
        b, qh = divmod(core, 2)
        qsl = slice(qh * qh_len, (qh + 1) * qh_len)
        am = align_mask[b, :, qsl]
        am_act = np.concatenate([am[kt * 128 : (kt + 1) * 128] for kt in ACT_KTS])
        am_dve = np.concatenate([am[kt * 128 : (kt + 1) * 128] for kt in DVE_KTS])
        m = {
            "xqT": np.ascontiguousarray(q_hidden_states[b, qsl].T),
            "xkT": np.ascontiguousarray(k_hidden_states[b].T),
            "amf": np.ascontiguousarray(am_act.astype(ml_dtypes.bfloat16)),
            "amq": np.ascontiguousarray(
                np.where(am_dve != 0, AMQ_KEEP, AMQ_KILL).astype(np.int16)
            ),
            "wqT": wq,
            "wkT": wk,
            "wvT": wv,
            "woT": wo,
        }
        in_maps.append(m)

    global _last_in_maps
    _last_in_maps = in_maps
    res = run_bass_kernel_spmd(nc, in_maps, list(range(8))).results
    out = np.empty((B, Q, HID), np.float32)
    for core in range(8):
        b, qh = divmod(core, 2)
        out[b, qh * qh_len : (qh + 1) * qh_len] = res[core]["out"]
    return out


# revision 26
# speedup vs baseline: 1.0222x; 1.0159x over previous
"""KgAdapterCrossAttention kernel for 8 trn2 NeuronCores.

Sharding: core = (batch b, query-half qh).  Each core computes attention for
1024 queries of one batch element against all 2048 keys.  221us -> 81us vs
the fp32 baseline (2.7x), all numerics within 1e-2 of the fp32 reference.

Design notes:
  - All matmuls use float32r / bf16 operands: 1 cycle/row on the PE instead
    of fp32's 4 (f32r keeps fp32 accuracy; inputs arrive as f32r via DMA,
    PSUM->SBUF copies act as the required f32r rounding ops).
  - Scores are computed pre-scaled by log2e*128 (folded into Wq on the host).
  - The 16 k-tiles of each (query-block, head) are processed in PAIRS that
    share a [128,1024] PSUM tile (two banks, sequential accumulation groups)
    so each exp instruction covers 1024 columns, amortizing the ~150ns
    access-latency overhead of Act/DVE instructions.  3 pair-slots rotate so
    both exp engines stream without stalling on PSUM.
  - exp splits across engines (pair schedule D,A,A,D,A,A,D,A): Act pairs use
    native Exp (scale=1/(log2e*128)) followed by a gpsimd align-mask
    multiply; DVE pairs use a fused fast-exp: ONE tensor_add of the PSUM
    scores with amq = mask ? 16250 : 8192 (i16), truncated to i16, whose
    bf16 BITCAST equals exp2(s*log2e) with the mask folded in (masked lanes
    land at ~2^-63).  16250 rather than 16256 centers the exponent-trick's
    linear-interpolation error (+-3%, zero mean, cancels in softmax).
  - P*V chains run per (qt, head); qt0 interleaves one head behind the score
    pipeline, qt1..3 defer into the NEXT query-block's head phases (pt tiles
    for heads 0/1 are double-buffered across blocks to allow the overlap).
    The softmax denominator rides along as a ones-column in V; normalize
    folds into the PSUM->SBUF copy as a per-partition scalar multiply.
  - O-projection transposes write into spare space of the O PSUM bank; the
    tail's qt2/3 accumulators borrow idle score-pool slots.
  - Mask DMAs are chunked per consumption unit and interleaved with the
    activation loads so arrivals pace the pipeline start.
"""

import os
import sys

import numpy as np
import ml_dtypes

try:
    import concourse.bass as bass
except ImportError:
    for _p in ("/opt/trn_rl_repo", os.path.expanduser("~/.axon_site/_ro/trn_rl_repo")):
        if os.path.isdir(_p) and _p not in sys.path:
            sys.path.insert(0, _p)
    import concourse.bass as bass

import concourse.mybir as mybir
import concourse.tile as tile
from concourse import bacc
from concourse.masks import make_identity
from contextlib import ExitStack

F32 = mybir.dt.float32
F32R = mybir.dt.float32r
BF16 = mybir.dt.bfloat16
I16 = mybir.dt.int16
EXP = mybir.ActivationFunctionType.Exp
ALU = mybir.AluOpType

P = 128
HID = 256
NHEAD = 4
DHEAD = 64
NQ = 1024  # queries per core
NK = 2048  # keys (full)
QBLK = 512
NQB = NQ // QBLK  # 2
NKT = NK // P  # 16
NPAIR = NKT // 2  # 8
NCT = HID // P  # 2

FE_MUL = float(np.log2(np.e)) * 128.0  # folded into Wq on host
ACT_SCALE = 1.0 / FE_MUL
AMQ_KEEP = 16250  # 127*128 minus centering delta 6
AMQ_KILL = 8192   # masked lanes -> bf16 2^-63 ~ 0

# engine per kt-pair: D = DVE fast-exp, A = Act exp (spread to avoid
# consecutive-A runs starving the 2-slot score-PSUM rotation)
PAIR_ENG = ["D", "A", "A", "D", "A", "A", "D", "A"]
ACT_KTS = [2, 3, 4, 5, 8, 9, 10, 11, 14, 15]
DVE_KTS = [0, 1, 6, 7, 12, 13]
ACT_POS = {kt: j for j, kt in enumerate(ACT_KTS)}
DVE_POS = {kt: j for j, kt in enumerate(DVE_KTS)}


def build() -> bass.Bass:
    nc = bacc.Bacc()
    xqT = nc.declare_dram_parameter("xqT", [HID, NQ], F32R, isOutput=False)
    xkT = nc.declare_dram_parameter("xkT", [HID, NK], F32R, isOutput=False)
    amf = nc.declare_dram_parameter("amf", [len(ACT_KTS) * P, NQ], BF16, isOutput=False)
    amq = nc.declare_dram_parameter("amq", [len(DVE_KTS) * P, NQ], I16, isOutput=False)
    wqT = nc.declare_dram_parameter("wqT", [HID, HID], F32R, isOutput=False)
    wkT = nc.declare_dram_parameter("wkT", [HID, HID], F32R, isOutput=False)
    wvT = nc.declare_dram_parameter("wvT", [HID, HID], F32R, isOutput=False)
    woT = nc.declare_dram_parameter("woT", [HID, HID], F32R, isOutput=False)
    out_d = nc.declare_dram_parameter("out", [NQ, HID], F32, isOutput=True)

    with tile.TileContext(nc) as tc, ExitStack() as ctx:
        const = ctx.enter_context(tc.tile_pool(name="const", bufs=1))
        big = ctx.enter_context(tc.tile_pool(name="big", bufs=1))
        ptp = ctx.enter_context(tc.tile_pool(name="ptp", bufs=1))
        amp = ctx.enter_context(tc.tile_pool(name="amp", bufs=1))
        wrk = ctx.enter_context(tc.tile_pool(name="wrk", bufs=2))
        wrk1 = ctx.enter_context(tc.tile_pool(name="wrk1", bufs=1))
        ps_st = ctx.enter_context(tc.tile_pool(name="ps_st", bufs=3, space="PSUM"))
        ps_a = ctx.enter_context(tc.tile_pool(name="ps_a", bufs=1, space="PSUM"))
        ps_o = ctx.enter_context(tc.tile_pool(name="ps_o", bufs=1, space="PSUM"))

        # --- DMA loads (the DMA device serializes; K side first) ---
        def load2(name, src, width, dt=F32R):
            ts = []
            for t in range(2):
                tl = const.tile([P, width], dt, tag=f"{name}{t}", name=f"{name}{t}")
                nc.sync.dma_start(out=tl, in_=src[t * P : (t + 1) * P, :])
                ts.append(tl)
            return ts

        wk_sb = load2("wk", wkT, HID)
        xk_sb = []
        for t in range(2):
            tl = big.tile([P, NK], F32R, tag=f"xk{t}", name=f"xk{t}")
            nc.sync.dma_start(out=tl, in_=xkT[t * P : (t + 1) * P, :])
            xk_sb.append(tl)
        wq_sb = load2("wq", wqT, HID)
        wv_sb = load2("wv", wvT, HID)
        xq_sb = []
        for t in range(2):
            tl = big.tile([P, NQ], F32R, tag=f"xq{t}", name=f"xq{t}")
            nc.sync.dma_start(out=tl, in_=xqT[t * P : (t + 1) * P, :])
            xq_sb.append(tl)

        amf_r = amf.rearrange("(t p) q -> p t q", p=P)
        amq_r = amq.rearrange("(t p) q -> p t q", p=P)
        am_sb = {}   # (qb, act_pos_start) -> tile [P, w, QBLK] bf16
        amq_sb = {}  # (qb, dve_pos_start) -> tile [P, w, QBLK] i16

        def load_mask_chunks(qb):
            # one chunk per consumption unit, in pair order, so the DMA
            # arrivals pace the pipeline
            qsl = slice(qb * QBLK, (qb + 1) * QBLK)
            a_pos = 0
            d_pos = 0
            for pj in range(NPAIR):
                eng = PAIR_ENG[pj]
                if eng in ("D", "S"):
                    w = 1 if eng == "S" else 2
                    tq = amp.tile(
                        [P, w, QBLK], I16, tag=f"amq_{d_pos}", name=f"amq_{d_pos}"
                    )
                    nc.sync.dma_start(out=tq, in_=amq_r[:, d_pos : d_pos + w, qsl])
                    amq_sb[(qb, d_pos)] = tq
                    d_pos += w
                if eng in ("A", "S"):
                    w = 1 if eng == "S" else 2
                    tl = amp.tile(
                        [P, w, QBLK], BF16, tag=f"am_{a_pos}", name=f"am_{a_pos}"
                    )
                    nc.sync.dma_start(out=tl, in_=amf_r[:, a_pos : a_pos + w, qsl])
                    am_sb[(qb, a_pos)] = tl
                    a_pos += w

        load_mask_chunks(0)
        wo_sb = load2("wo", woT, HID)

        ident_f = const.tile([P, P], F32, tag="ident_f", name="ident_f")
        make_identity(nc, ident_f)
        ident = const.tile([P, P], F32R, tag="ident", name="ident")
        nc.vector.tensor_copy(ident, ident_f)

        # --- projections (copies alternate Act/DVE; prologue work) ---
        cp_i = 0

        def copy_eng(out, in_):
            nonlocal cp_i
            cp_i += 1
            if cp_i % 2 == 0:
                nc.scalar.copy(out, in_)
            else:
                nc.vector.tensor_copy(out, in_)

        kt_sb = [big.tile([P, NK], F32R, tag=f"kt{t}", name=f"kt{t}") for t in range(2)]
        for t in range(2):
            for nb in range(NK // (2 * QBLK)):
                ps = ps_st.tile([P, 2 * QBLK], F32, tag="st", name="st")
                for half in range(2):
                    for ct in range(NCT):
                        nc.tensor.matmul(
                            ps[:, half * QBLK : (half + 1) * QBLK],
                            lhsT=wk_sb[ct][:, t * P : (t + 1) * P],
                            rhs=xk_sb[ct][
                                :, (2 * nb + half) * QBLK : (2 * nb + half + 1) * QBLK
                            ],
                            start=(ct == 0),
                            stop=(ct == NCT - 1),
                        )
                copy_eng(kt_sb[t][:, 2 * nb * QBLK : (2 * nb + 2) * QBLK], ps)

        qt_sb = [big.tile([P, NQ], F32R, tag=f"qt{t}", name=f"qt{t}") for t in range(2)]
        for t in range(2):
            ps = ps_st.tile([P, 2 * QBLK], F32, tag="st", name="st")
            for half in range(2):
                for ct in range(NCT):
                    nc.tensor.matmul(
                        ps[:, half * QBLK : (half + 1) * QBLK],
                        lhsT=wq_sb[ct][:, t * P : (t + 1) * P],
                        rhs=xq_sb[ct][:, half * QBLK : (half + 1) * QBLK],
                        start=(ct == 0),
                        stop=(ct == NCT - 1),
                    )
            copy_eng(qt_sb[t], ps)

        # V''[ktok, h, 0:64] = V rows (bf16); V''[ktok, h, 64] = 1.0
        vpp = [None] * NKT
        for kt2 in range(NKT // 2):
            ps = ps_st.tile([P, 2 * QBLK], F32, tag="st", name="st")
            for half in range(2):
                kt = 2 * kt2 + half
                for ct in range(NCT):
                    nc.tensor.matmul(
                        ps[:, half * QBLK : half * QBLK + HID],
                        lhsT=xk_sb[ct][:, kt * P : (kt + 1) * P],
                        rhs=wv_sb[ct],
                        start=(ct == 0),
                        stop=(ct == NCT - 1),
                    )
            for half in range(2):
                kt = 2 * kt2 + half
                tl = big.tile([P, NHEAD, DHEAD + 1], BF16, tag=f"v{kt}", name=f"v{kt}")
                copy_eng(
                    tl[:, :, 0:DHEAD],
                    ps[:, half * QBLK : half * QBLK + HID].rearrange(
                        "p (h d) -> p h d", h=NHEAD
                    ),
                )
                nc.gpsimd.memset(tl[:, :, DHEAD : DHEAD + 1], 1.0)
                vpp[kt] = tl

        # --- attention ---
        def emit_head(qb, h, pts):
            qsl = slice(qb * QBLK, (qb + 1) * QBLK)
            t, po = h // 2, (h % 2) * DHEAD
            for pj in range(NPAIR):
                ps = ps_st.tile([P, 2 * QBLK], F32, tag="st", name="st")
                for half in range(2):
                    kt = 2 * pj + half
                    nc.tensor.matmul(
                        ps[:, half * QBLK : (half + 1) * QBLK],
                        lhsT=kt_sb[t][po : po + DHEAD, kt * P : (kt + 1) * P],
                        rhs=qt_sb[t][po : po + DHEAD, qsl],
                        start=True,
                        stop=True,
                    )
                eng = PAIR_ENG[pj]
                tg = f"pt{h}_{pj}" if h > 2 else f"pt{h}{qb % 2}_{pj}"
                if eng == "A":
                    pt = ptp.tile([P, 2 * QBLK], BF16, tag=tg, name=tg)
                    nc.scalar.activation(pt, ps, EXP, scale=ACT_SCALE)
                    j = ACT_POS[2 * pj]
                    nc.gpsimd.tensor_mul(
                        pt.rearrange("p (two q) -> p two q", two=2),
                        pt.rearrange("p (two q) -> p two q", two=2),
                        am_sb[(qb, j)],
                    )
                    for half in range(2):
                        pts[(h, 2 * pj + half)] = pt[:, half * QBLK : (half + 1) * QBLK]
                elif eng == "D":
                    ie = ptp.tile([P, 2 * QBLK], I16, tag=tg, name=tg)
                    j = DVE_POS[2 * pj]
                    nc.vector.tensor_add(
                        ie.rearrange("p (two q) -> p two q", two=2),
                        ps.rearrange("p (two q) -> p two q", two=2),
                        amq_sb[(qb, j)],
                    )
                    pv = ie.bitcast(BF16)
                    for half in range(2):
                        pts[(h, 2 * pj + half)] = pv[:, half * QBLK : (half + 1) * QBLK]
                else:  # split pair: kt14 -> DVE, kt15 -> Act
                    ie = ptp.tile([P, QBLK], I16, tag=f"pti{h}_{pj}", name=f"pti{h}_{pj}")
                    jd = DVE_POS[2 * pj]
                    nc.vector.tensor_add(
                        ie, ps[:, 0:QBLK], amq_sb[(qb, jd)][:, 0, :]
                    )
                    pts[(h, 2 * pj)] = ie.bitcast(BF16)
                    pt = ptp.tile([P, QBLK], BF16, tag=f"pta{h}_{pj}", name=f"pta{h}_{pj}")
                    nc.scalar.activation(pt, ps[:, QBLK : 2 * QBLK], EXP, scale=ACT_SCALE)
                    ja = ACT_POS[2 * pj + 1]
                    nc.gpsimd.tensor_mul(pt, pt, am_sb[(qb, ja)][:, 0, :])
                    pts[(h, 2 * pj + 1)] = pt

        deferred = []

        def run_deferred():
            if deferred:
                deferred.pop(0)()

        def process_qb(qb):
            if qb > 0:
                load_mask_chunks(qb)
            pts = {}
            ps_avs = {}

            def get_ps_av(qt):
                if qt not in ps_avs:
                    if qt < 2:
                        ps_avs[qt] = ps_a.tile(
                            [P, NHEAD * (DHEAD + 1)], F32, tag="a0",
                            name="a0", padded_shape=[P, QBLK],
                        )
                    else:
                        # tail-only: borrow an idle score-pool slot
                        big_t = ps_st.tile([P, 2 * QBLK], F32, tag="st", name="st_a")
                        ps_avs[qt] = big_t[:, 0 : NHEAD * (DHEAD + 1)]
                return ps_avs[qt]

            def emit_chain(qt, h):
                ps_av = get_ps_av(qt)
                for kt in range(NKT):
                    nc.tensor.matmul(
                        ps_av[:, h * 65 : (h + 1) * 65],
                        lhsT=pts[(h, kt)][:, qt * P : (qt + 1) * P],
                        rhs=vpp[kt][:, h, :],
                        start=(kt == 0),
                        stop=(kt == NKT - 1),
                    )

            def emit_norm(qt):
                ps_av = ps_avs.pop(qt)
                rec = wrk.tile([P, NHEAD], F32, tag=f"rec{qt % 2}", name=f"rec{qt % 2}")
                nc.vector.reciprocal(rec, ps_av[:, DHEAD : NHEAD * 65 : 65])
                an = wrk1.tile([P, HID], F32R, tag=f"an{qt % 2}", name=f"an{qt % 2}")
                for h in range(NHEAD):
                    nc.vector.tensor_scalar_mul(
                        an[:, h * DHEAD : (h + 1) * DHEAD],
                        ps_av[:, h * 65 : h * 65 + DHEAD],
                        rec[:, h : h + 1],
                    )
                return an

            def emit_o(qt, an):
                o_ps = ps_o.tile([P, QBLK], F32, tag="o", name="o_ps")
                atts = []
                for ct in range(NCT):
                    tp = o_ps[:, HID + ct * P : HID + (ct + 1) * P].bitcast(F32R)
                    nc.tensor.transpose(tp, an[:, ct * P : (ct + 1) * P], ident)
                    att = wrk.tile([P, P], F32R, tag=f"att{ct}", name=f"att{ct}")
                    nc.vector.tensor_copy(att, tp)
                    atts.append(att)
                for ct in range(NCT):
                    nc.tensor.matmul(
                        o_ps[:, 0:HID],
                        lhsT=atts[ct],
                        rhs=wo_sb[ct],
                        start=(ct == 0),
                        stop=(ct == NCT - 1),
                    )
                ob = wrk1.tile([P, HID], F32, tag=f"ob{qt % 2}", name=f"ob{qt % 2}")
                nc.vector.tensor_copy(ob, o_ps[:, 0:HID])
                q0 = qb * QBLK + qt * P
                nc.sync.dma_start(out=out_d[q0 : q0 + P, :], in_=ob)

            for h in range(NHEAD):
                emit_head(qb, h, pts)
                if h in (1, 2):
                    # previous qb's deferred tail chains/outputs overlap
                    # this qb's first head phases (two slices to spread the
                    # extra PE load)
                    run_deferred()
                if h > 0:
                    emit_chain(0, h - 1)
            emit_chain(0, NHEAD - 1)
            an0 = emit_norm(0)

            last = qb == NQB - 1

            def tail1(emit_chain=emit_chain, emit_norm=emit_norm,
                      emit_o=emit_o, an0=an0):
                emit_o(0, an0)
                for h in range(NHEAD):
                    emit_chain(1, h)
                an1 = emit_norm(1)
                emit_o(1, an1)

            def tail2(emit_chain=emit_chain, emit_norm=emit_norm,
                      emit_o=emit_o):
                for h in range(NHEAD):
                    emit_chain(2, h)
                for h in range(NHEAD):
                    emit_chain(3, h)
                an2 = emit_norm(2)
                emit_o(2, an2)
                an3 = emit_norm(3)
                emit_o(3, an3)

            deferred.append(tail1)
            deferred.append(tail2)

        for qb in range(NQB):
            process_qb(qb)
        run_deferred()
        run_deferred()
    nc.compile()
    return nc


_NC_CACHE = {}
_last_in_maps = None


def _get_nc(with_attn_mask: bool = False) -> bass.Bass:
    key = "v5"
    if key not in _NC_CACHE:
        _NC_CACHE[key] = build()
    return _NC_CACHE[key]


def kernel(q_hidden_states, k_hidden_states, attention_mask, align_mask, Wq, Wk, Wv, Wo):
    from concourse.bass_utils import run_bass_kernel_spmd

    q_hidden_states = np.asarray(q_hidden_states, np.float32)
    k_hidden_states = np.asarray(k_hidden_states, np.float32)
    attention_mask = np.asarray(attention_mask, np.float32)
    align_mask = np.asarray(align_mask)
    B, Q, _ = q_hidden_states.shape
    qh_len = Q // 2  # 1024

    nc = _get_nc()

    # scores arrive in PSUM pre-scaled by log2e*128 (folded into Wq here)
    wq = np.ascontiguousarray(np.asarray(Wq, np.float32).T) * np.float32(FE_MUL / 8.0)
    wk = np.ascontiguousarray(np.asarray(Wk, np.float32).T)
    wv = np.ascontiguousarray(np.asarray(Wv, np.float32).T)
    wo = np.ascontiguousarray(np.asarray(Wo, np.float32).T)

    if np.any(attention_mask):
        raise NotImplementedError("nonzero additive attention_mask not supported")

    in_maps = []
    for core in range(8):
        b, qh = divmod(core, 2)
        qsl = slice(qh * qh_len, (qh + 1) * qh_len)
        am = align_mask[b, :, qsl]
        am_act = np.concatenate([am[kt * 128 : (kt + 1) * 128] for kt in ACT_KTS])
        am_dve = np.concatenate([am[kt * 128 : (kt + 1) * 128] for kt in DVE_KTS])
        m = {
            "xqT": np.ascontiguousarray(q_hidden_states[b, qsl].T),
            "xkT": np.ascontiguousarray(k_hidden_states[b].T),
            "amf": np.ascontiguousarray(am_act.astype(ml_dtypes.bfloat16)),
            "amq": np.ascontiguousarray(
                np.where(am_dve != 0, AMQ_KEEP, AMQ_KILL).astype(np.int16)
            ),
            "wqT": wq,
            "wkT": wk,
            "wvT": wv,
            "woT": wo,
        }
        in_maps.append(m)

    global _last_in_maps
    _last_in_maps = in_maps
    res = run_bass_kernel_spmd(nc, in_maps, list(range(8))).results
    out = np.empty((B, Q, HID), np.float32)
    for core in range(8):
        b, qh = divmod(core, 2)
        out[b, qh * qh_len : (qh + 1) * qh_len] = res[core]["out"]
    return out


# revision 27
# speedup vs baseline: 1.0296x; 1.0073x over previous
"""KgAdapterCrossAttention kernel for 8 trn2 NeuronCores.

Sharding: core = (batch b, query-half qh).  Each core computes attention for
1024 queries of one batch element against all 2048 keys.  221us -> 81us vs
the fp32 baseline (2.7x), all numerics within 1e-2 of the fp32 reference.

Design notes:
  - All matmuls use float32r / bf16 operands: 1 cycle/row on the PE instead
    of fp32's 4 (f32r keeps fp32 accuracy; inputs arrive as f32r via DMA,
    PSUM->SBUF copies act as the required f32r rounding ops).
  - Scores are computed pre-scaled by log2e*128 (folded into Wq on the host).
  - The 16 k-tiles of each (query-block, head) are processed in PAIRS that
    share a [128,1024] PSUM tile (two banks, sequential accumulation groups)
    so each exp instruction covers 1024 columns, amortizing the ~150ns
    access-latency overhead of Act/DVE instructions.  3 pair-slots rotate so
    both exp engines stream without stalling on PSUM.
  - exp splits across engines (pair schedule D,A,A,D,A,A,D,A): Act pairs use
    native Exp (scale=1/(log2e*128)) followed by a gpsimd align-mask
    multiply; DVE pairs use a fused fast-exp: ONE tensor_add of the PSUM
    scores with amq = mask ? 16250 : 8192 (i16), truncated to i16, whose
    bf16 BITCAST equals exp2(s*log2e) with the mask folded in (masked lanes
    land at ~2^-63).  16250 rather than 16256 centers the exponent-trick's
    linear-interpolation error (+-3%, zero mean, cancels in softmax).
  - P*V chains run per (qt, head); qt0 interleaves one head behind the score
    pipeline, qt1..3 defer into the NEXT query-block's head phases (pt tiles
    for heads 0/1 are double-buffered across blocks to allow the overlap).
    The softmax denominator rides along as a ones-column in V; normalize
    folds into the PSUM->SBUF copy as a per-partition scalar multiply.
  - O-projection transposes write into spare space of the O PSUM bank; the
    tail's qt2/3 accumulators borrow idle score-pool slots.
  - Mask DMAs are chunked per consumption unit and interleaved with the
    activation loads so arrivals pace the pipeline start.
"""

import os
import sys

import numpy as np
import ml_dtypes

try:
    import concourse.bass as bass
except ImportError:
    for _p in ("/opt/trn_rl_repo", os.path.expanduser("~/.axon_site/_ro/trn_rl_repo")):
        if os.path.isdir(_p) and _p not in sys.path:
            sys.path.insert(0, _p)
    import concourse.bass as bass

import concourse.mybir as mybir
import concourse.tile as tile
from concourse import bacc
from concourse.masks import make_identity
from contextlib import ExitStack

F32 = mybir.dt.float32
F32R = mybir.dt.float32r
BF16 = mybir.dt.bfloat16
I16 = mybir.dt.int16
EXP = mybir.ActivationFunctionType.Exp
ALU = mybir.AluOpType

P = 128
HID = 256
NHEAD = 4
DHEAD = 64
NQ = 1024  # queries per core
NK = 2048  # keys (full)
QBLK = 512
NQB = NQ // QBLK  # 2
NKT = NK // P  # 16
NPAIR = NKT // 2  # 8
NCT = HID // P  # 2

FE_MUL = float(np.log2(np.e)) * 128.0  # folded into Wq on host
ACT_SCALE = 1.0 / FE_MUL
AMQ_KEEP = 16250  # 127*128 minus centering delta 6
AMQ_KILL = 8192   # masked lanes -> bf16 2^-63 ~ 0

# engine per kt-pair: D = DVE fast-exp, A = Act exp (spread to avoid
# consecutive-A runs starving the 2-slot score-PSUM rotation)
PAIR_ENG = ["D", "A", "A", "D", "A", "A", "D", "A"]
ACT_KTS = [2, 3, 4, 5, 8, 9, 10, 11, 14, 15]
DVE_KTS = [0, 1, 6, 7, 12, 13]
ACT_POS = {kt: j for j, kt in enumerate(ACT_KTS)}
DVE_POS = {kt: j for j, kt in enumerate(DVE_KTS)}


def build() -> bass.Bass:
    nc = bacc.Bacc()
    xqT = nc.declare_dram_parameter("xqT", [HID, NQ], BF16, isOutput=False)
    xkT = nc.declare_dram_parameter("xkT", [HID, NK], F32R, isOutput=False)
    amf = nc.declare_dram_parameter("amf", [len(ACT_KTS) * P, NQ], BF16, isOutput=False)
    amq = nc.declare_dram_parameter("amq", [len(DVE_KTS) * P, NQ], I16, isOutput=False)
    wqT = nc.declare_dram_parameter("wqT", [HID, HID], BF16, isOutput=False)
    wkT = nc.declare_dram_parameter("wkT", [HID, HID], F32R, isOutput=False)
    wvT = nc.declare_dram_parameter("wvT", [HID, HID], F32R, isOutput=False)
    woT = nc.declare_dram_parameter("woT", [HID, HID], F32R, isOutput=False)
    out_d = nc.declare_dram_parameter("out", [NQ, HID], F32, isOutput=True)

    with tile.TileContext(nc) as tc, ExitStack() as ctx:
        const = ctx.enter_context(tc.tile_pool(name="const", bufs=1))
        big = ctx.enter_context(tc.tile_pool(name="big", bufs=1))
        ptp = ctx.enter_context(tc.tile_pool(name="ptp", bufs=1))
        amp = ctx.enter_context(tc.tile_pool(name="amp", bufs=1))
        wrk = ctx.enter_context(tc.tile_pool(name="wrk", bufs=2))
        wrk1 = ctx.enter_context(tc.tile_pool(name="wrk1", bufs=1))
        ps_st = ctx.enter_context(tc.tile_pool(name="ps_st", bufs=3, space="PSUM"))
        ps_a = ctx.enter_context(tc.tile_pool(name="ps_a", bufs=1, space="PSUM"))
        ps_o = ctx.enter_context(tc.tile_pool(name="ps_o", bufs=1, space="PSUM"))

        # --- DMA loads (the DMA device serializes; K side first) ---
        def load2(name, src, width, dt=F32R):
            ts = []
            for t in range(2):
                tl = const.tile([P, width], dt, tag=f"{name}{t}", name=f"{name}{t}")
                nc.sync.dma_start(out=tl, in_=src[t * P : (t + 1) * P, :])
                ts.append(tl)
            return ts

        wk_sb = load2("wk", wkT, HID)
        xk_sb = []
        for t in range(2):
            tl = big.tile([P, NK], F32R, tag=f"xk{t}", name=f"xk{t}")
            nc.sync.dma_start(out=tl, in_=xkT[t * P : (t + 1) * P, :])
            xk_sb.append(tl)
        wq_sb = load2("wq", wqT, HID, dt=BF16)
        wv_sb = load2("wv", wvT, HID)
        xq_sb = []
        for t in range(2):
            tl = big.tile([P, NQ], BF16, tag=f"xq{t}", name=f"xq{t}")
            nc.sync.dma_start(out=tl, in_=xqT[t * P : (t + 1) * P, :])
            xq_sb.append(tl)

        amf_r = amf.rearrange("(t p) q -> p t q", p=P)
        amq_r = amq.rearrange("(t p) q -> p t q", p=P)
        am_sb = {}   # (qb, act_pos_start) -> tile [P, w, QBLK] bf16
        amq_sb = {}  # (qb, dve_pos_start) -> tile [P, w, QBLK] i16

        def load_mask_chunks(qb):
            # one chunk per consumption unit, in pair order, so the DMA
            # arrivals pace the pipeline
            qsl = slice(qb * QBLK, (qb + 1) * QBLK)
            a_pos = 0
            d_pos = 0
            for pj in range(NPAIR):
                eng = PAIR_ENG[pj]
                if eng in ("D", "S"):
                    w = 1 if eng == "S" else 2
                    tq = amp.tile(
                        [P, w, QBLK], I16, tag=f"amq_{d_pos}", name=f"amq_{d_pos}"
                    )
                    nc.sync.dma_start(out=tq, in_=amq_r[:, d_pos : d_pos + w, qsl])
                    amq_sb[(qb, d_pos)] = tq
                    d_pos += w
                if eng in ("A", "S"):
                    w = 1 if eng == "S" else 2
                    tl = amp.tile(
                        [P, w, QBLK], BF16, tag=f"am_{a_pos}", name=f"am_{a_pos}"
                    )
                    nc.sync.dma_start(out=tl, in_=amf_r[:, a_pos : a_pos + w, qsl])
                    am_sb[(qb, a_pos)] = tl
                    a_pos += w

        load_mask_chunks(0)
        wo_sb = load2("wo", woT, HID)

        ident_f = const.tile([P, P], F32, tag="ident_f", name="ident_f")
        make_identity(nc, ident_f)
        ident = const.tile([P, P], F32R, tag="ident", name="ident")
        nc.vector.tensor_copy(ident, ident_f)

        # --- projections (copies alternate Act/DVE; prologue work) ---
        cp_i = 0

        def copy_eng(out, in_):
            nonlocal cp_i
            cp_i += 1
            if cp_i % 2 == 0:
                nc.scalar.copy(out, in_)
            else:
                nc.vector.tensor_copy(out, in_)

        kt_sb = [big.tile([P, NK], F32R, tag=f"kt{t}", name=f"kt{t}") for t in range(2)]
        for t in range(2):
            for nb in range(NK // (2 * QBLK)):
                ps = ps_st.tile([P, 2 * QBLK], F32, tag="st", name="st")
                for half in range(2):
                    for ct in range(NCT):
                        nc.tensor.matmul(
                            ps[:, half * QBLK : (half + 1) * QBLK],
                            lhsT=wk_sb[ct][:, t * P : (t + 1) * P],
                            rhs=xk_sb[ct][
                                :, (2 * nb + half) * QBLK : (2 * nb + half + 1) * QBLK
                            ],
                            start=(ct == 0),
                            stop=(ct == NCT - 1),
                        )
                copy_eng(kt_sb[t][:, 2 * nb * QBLK : (2 * nb + 2) * QBLK], ps)

        qt_sb = [big.tile([P, NQ], F32R, tag=f"qt{t}", name=f"qt{t}") for t in range(2)]
        for t in range(2):
            ps = ps_st.tile([P, 2 * QBLK], F32, tag="st", name="st")
            for half in range(2):
                for ct in range(NCT):
                    nc.tensor.matmul(
                        ps[:, half * QBLK : (half + 1) * QBLK],
                        lhsT=wq_sb[ct][:, t * P : (t + 1) * P],
                        rhs=xq_sb[ct][:, half * QBLK : (half + 1) * QBLK],
                        start=(ct == 0),
                        stop=(ct == NCT - 1),
                    )
            copy_eng(qt_sb[t], ps)

        # V''[ktok, h, 0:64] = V rows (bf16); V''[ktok, h, 64] = 1.0
        vpp = [None] * NKT
        for kt2 in range(NKT // 2):
            ps = ps_st.tile([P, 2 * QBLK], F32, tag="st", name="st")
            for half in range(2):
                kt = 2 * kt2 + half
                for ct in range(NCT):
                    nc.tensor.matmul(
                        ps[:, half * QBLK : half * QBLK + HID],
                        lhsT=xk_sb[ct][:, kt * P : (kt + 1) * P],
                        rhs=wv_sb[ct],
                        start=(ct == 0),
                        stop=(ct == NCT - 1),
                    )
            for half in range(2):
                kt = 2 * kt2 + half
                tl = big.tile([P, NHEAD, DHEAD + 1], BF16, tag=f"v{kt}", name=f"v{kt}")
                copy_eng(
                    tl[:, :, 0:DHEAD],
                    ps[:, half * QBLK : half * QBLK + HID].rearrange(
                        "p (h d) -> p h d", h=NHEAD
                    ),
                )
                nc.gpsimd.memset(tl[:, :, DHEAD : DHEAD + 1], 1.0)
                vpp[kt] = tl

        # --- attention ---
        def emit_head(qb, h, pts):
            qsl = slice(qb * QBLK, (qb + 1) * QBLK)
            t, po = h // 2, (h % 2) * DHEAD
            for pj in range(NPAIR):
                ps = ps_st.tile([P, 2 * QBLK], F32, tag="st", name="st")
                for half in range(2):
                    kt = 2 * pj + half
                    nc.tensor.matmul(
                        ps[:, half * QBLK : (half + 1) * QBLK],
                        lhsT=kt_sb[t][po : po + DHEAD, kt * P : (kt + 1) * P],
                        rhs=qt_sb[t][po : po + DHEAD, qsl],
                        start=True,
                        stop=True,
                    )
                eng = PAIR_ENG[pj]
                tg = f"pt{h}_{pj}" if h > 2 else f"pt{h}{qb % 2}_{pj}"
                if eng == "A":
                    pt = ptp.tile([P, 2 * QBLK], BF16, tag=tg, name=tg)
                    nc.scalar.activation(pt, ps, EXP, scale=ACT_SCALE)
                    j = ACT_POS[2 * pj]
                    nc.gpsimd.tensor_mul(
                        pt.rearrange("p (two q) -> p two q", two=2),
                        pt.rearrange("p (two q) -> p two q", two=2),
                        am_sb[(qb, j)],
                    )
                    for half in range(2):
                        pts[(h, 2 * pj + half)] = pt[:, half * QBLK : (half + 1) * QBLK]
                elif eng == "D":
                    ie = ptp.tile([P, 2 * QBLK], I16, tag=tg, name=tg)
                    j = DVE_POS[2 * pj]
                    nc.vector.tensor_add(
                        ie.rearrange("p (two q) -> p two q", two=2),
                        ps.rearrange("p (two q) -> p two q", two=2),
                        amq_sb[(qb, j)],
                    )
                    pv = ie.bitcast(BF16)
                    for half in range(2):
                        pts[(h, 2 * pj + half)] = pv[:, half * QBLK : (half + 1) * QBLK]
                else:  # split pair: kt14 -> DVE, kt15 -> Act
                    ie = ptp.tile([P, QBLK], I16, tag=f"pti{h}_{pj}", name=f"pti{h}_{pj}")
                    jd = DVE_POS[2 * pj]
                    nc.vector.tensor_add(
                        ie, ps[:, 0:QBLK], amq_sb[(qb, jd)][:, 0, :]
                    )
                    pts[(h, 2 * pj)] = ie.bitcast(BF16)
                    pt = ptp.tile([P, QBLK], BF16, tag=f"pta{h}_{pj}", name=f"pta{h}_{pj}")
                    nc.scalar.activation(pt, ps[:, QBLK : 2 * QBLK], EXP, scale=ACT_SCALE)
                    ja = ACT_POS[2 * pj + 1]
                    nc.gpsimd.tensor_mul(pt, pt, am_sb[(qb, ja)][:, 0, :])
                    pts[(h, 2 * pj + 1)] = pt

        deferred = []

        def run_deferred():
            if deferred:
                deferred.pop(0)()

        def process_qb(qb):
            if qb > 0:
                load_mask_chunks(qb)
            pts = {}
            ps_avs = {}

            def get_ps_av(qt):
                if qt not in ps_avs:
                    if qt < 2:
                        ps_avs[qt] = ps_a.tile(
                            [P, NHEAD * (DHEAD + 1)], F32, tag="a0",
                            name="a0", padded_shape=[P, QBLK],
                        )
                    else:
                        # tail-only: borrow an idle score-pool slot
                        big_t = ps_st.tile([P, 2 * QBLK], F32, tag="st", name="st_a")
                        ps_avs[qt] = big_t[:, 0 : NHEAD * (DHEAD + 1)]
                return ps_avs[qt]

            def emit_chain(qt, h):
                ps_av = get_ps_av(qt)
                for kt in range(NKT):
                    nc.tensor.matmul(
                        ps_av[:, h * 65 : (h + 1) * 65],
                        lhsT=pts[(h, kt)][:, qt * P : (qt + 1) * P],
                        rhs=vpp[kt][:, h, :],
                        start=(kt == 0),
                        stop=(kt == NKT - 1),
                    )

            def emit_norm(qt):
                ps_av = ps_avs.pop(qt)
                rec = wrk.tile([P, NHEAD], F32, tag=f"rec{qt % 2}", name=f"rec{qt % 2}")
                nc.vector.reciprocal(rec, ps_av[:, DHEAD : NHEAD * 65 : 65])
                an = wrk1.tile([P, HID], F32R, tag=f"an{qt % 2}", name=f"an{qt % 2}")
                for h in range(NHEAD):
                    nc.vector.tensor_scalar_mul(
                        an[:, h * DHEAD : (h + 1) * DHEAD],
                        ps_av[:, h * 65 : h * 65 + DHEAD],
                        rec[:, h : h + 1],
                    )
                return an

            def emit_o(qt, an):
                o_ps = ps_o.tile([P, QBLK], F32, tag="o", name="o_ps")
                atts = []
                for ct in range(NCT):
                    tp = o_ps[:, HID + ct * P : HID + (ct + 1) * P].bitcast(F32R)
                    nc.tensor.transpose(tp, an[:, ct * P : (ct + 1) * P], ident)
                    att = wrk.tile([P, P], F32R, tag=f"att{ct}", name=f"att{ct}")
                    nc.vector.tensor_copy(att, tp)
                    atts.append(att)
                for ct in range(NCT):
                    nc.tensor.matmul(
                        o_ps[:, 0:HID],
                        lhsT=atts[ct],
                        rhs=wo_sb[ct],
                        start=(ct == 0),
                        stop=(ct == NCT - 1),
                    )
                ob = wrk1.tile([P, HID], F32, tag=f"ob{qt % 2}", name=f"ob{qt % 2}")
                nc.vector.tensor_copy(ob, o_ps[:, 0:HID])
                q0 = qb * QBLK + qt * P
                nc.sync.dma_start(out=out_d[q0 : q0 + P, :], in_=ob)

            for h in range(NHEAD):
                emit_head(qb, h, pts)
                if h in (1, 2):
                    # previous qb's deferred tail chains/outputs overlap
                    # this qb's first head phases (two slices to spread the
                    # extra PE load)
                    run_deferred()
                if h > 0:
                    emit_chain(0, h - 1)
            emit_chain(0, NHEAD - 1)
            an0 = emit_norm(0)

            last = qb == NQB - 1

            def tail1(emit_chain=emit_chain, emit_norm=emit_norm,
                      emit_o=emit_o, an0=an0):
                emit_o(0, an0)
                for h in range(NHEAD):
                    emit_chain(1, h)
                an1 = emit_norm(1)
                emit_o(1, an1)

            def tail2(emit_chain=emit_chain, emit_norm=emit_norm,
                      emit_o=emit_o):
                for h in range(NHEAD):
                    emit_chain(2, h)
                an2 = emit_norm(2)
                for h in range(NHEAD):
                    emit_chain(3, h)
                emit_o(2, an2)
                an3 = emit_norm(3)
                emit_o(3, an3)

            deferred.append(tail1)
            deferred.append(tail2)

        for qb in range(NQB):
            process_qb(qb)
        run_deferred()
        run_deferred()
    nc.compile()
    return nc


_NC_CACHE = {}
_last_in_maps = None


def _get_nc(with_attn_mask: bool = False) -> bass.Bass:
    key = "v5"
    if key not in _NC_CACHE:
        _NC_CACHE[key] = build()
    return _NC_CACHE[key]


def kernel(q_hidden_states, k_hidden_states, attention_mask, align_mask, Wq, Wk, Wv, Wo):
    from concourse.bass_utils import run_bass_kernel_spmd

    q_hidden_states = np.asarray(q_hidden_states, np.float32)
    k_hidden_states = np.asarray(k_hidden_states, np.float32)
    attention_mask = np.asarray(attention_mask, np.float32)
    align_mask = np.asarray(align_mask)
    B, Q, _ = q_hidden_states.shape
    qh_len = Q // 2  # 1024

    nc = _get_nc()

    # scores arrive in PSUM pre-scaled by log2e*128 (folded into Wq here)
    wq = np.ascontiguousarray(
        (np.asarray(Wq, np.float32).T * np.float32(FE_MUL / 8.0)).astype(ml_dtypes.bfloat16)
    )
    wk = np.ascontiguousarray(np.asarray(Wk, np.float32).T)
    wv = np.ascontiguousarray(np.asarray(Wv, np.float32).T)
    wo = np.ascontiguousarray(np.asarray(Wo, np.float32).T)

    if np.any(attention_mask):
        raise NotImplementedError("nonzero additive attention_mask not supported")

    in_maps = []
    for core in range(8):
        b, qh = divmod(core, 2)
        qsl = slice(qh * qh_len, (qh + 1) * qh_len)
        am = align_mask[b, :, qsl]
        am_act = np.concatenate([am[kt * 128 : (kt + 1) * 128] for kt in ACT_KTS])
        am_dve = np.concatenate([am[kt * 128 : (kt + 1) * 128] for kt in DVE_KTS])
        m = {
            "xqT": np.ascontiguousarray(q_hidden_states[b, qsl].T.astype(ml_dtypes.bfloat16)),
            "xkT": np.ascontiguousarray(k_hidden_states[b].T),
            "amf": np.ascontiguousarray(am_act.astype(ml_dtypes.bfloat16)),
            "amq": np.ascontiguousarray(
                np.where(am_dve != 0, AMQ_KEEP, AMQ_KILL).astype(np.int16)
            ),
            "wqT": wq,
            "wkT": wk,
            "wvT": wv,
            "woT": wo,
        }
        in_maps.append(m)

    global _last_in_maps
    _last_in_maps = in_maps
    res = run_bass_kernel_spmd(nc, in_maps, list(range(8))).results
    out = np.empty((B, Q, HID), np.float32)
    for core in range(8):
        b, qh = divmod(core, 2)
        out[b, qh * qh_len : (qh + 1) * qh_len] = res[core]["out"]
    return out


# revision 28
# speedup vs baseline: 1.0506x; 1.0204x over previous
"""KgAdapterCrossAttention kernel for 8 trn2 NeuronCores.

Sharding: core = (batch b, query-half qh).  Each core computes attention for
1024 queries of one batch element against all 2048 keys.  221us -> 81us vs
the fp32 baseline (2.7x), all numerics within 1e-2 of the fp32 reference.

Design notes:
  - All matmuls use float32r / bf16 operands: 1 cycle/row on the PE instead
    of fp32's 4 (f32r keeps fp32 accuracy; inputs arrive as f32r via DMA,
    PSUM->SBUF copies act as the required f32r rounding ops).
  - Scores are computed pre-scaled by log2e*128 (folded into Wq on the host).
  - The 16 k-tiles of each (query-block, head) are processed in PAIRS that
    share a [128,1024] PSUM tile (two banks, sequential accumulation groups)
    so each exp instruction covers 1024 columns, amortizing the ~150ns
    access-latency overhead of Act/DVE instructions.  3 pair-slots rotate so
    both exp engines stream without stalling on PSUM.
  - exp splits across engines (pair schedule D,A,A,D,A,A,D,A): Act pairs use
    native Exp (scale=1/(log2e*128)) followed by a gpsimd align-mask
    multiply; DVE pairs use a fused fast-exp: ONE tensor_add of the PSUM
    scores with amq = mask ? 16250 : 8192 (i16), truncated to i16, whose
    bf16 BITCAST equals exp2(s*log2e) with the mask folded in (masked lanes
    land at ~2^-63).  16250 rather than 16256 centers the exponent-trick's
    linear-interpolation error (+-3%, zero mean, cancels in softmax).
  - P*V chains run per (qt, head); qt0 interleaves one head behind the score
    pipeline, qt1..3 defer into the NEXT query-block's head phases (pt tiles
    for heads 0/1 are double-buffered across blocks to allow the overlap).
    The softmax denominator rides along as a ones-column in V; normalize
    folds into the PSUM->SBUF copy as a per-partition scalar multiply.
  - O-projection transposes write into spare space of the O PSUM bank; the
    tail's qt2/3 accumulators borrow idle score-pool slots.
  - Mask DMAs are chunked per consumption unit and interleaved with the
    activation loads so arrivals pace the pipeline start.
"""

import os
import sys

import numpy as np
import ml_dtypes

try:
    import concourse.bass as bass
except ImportError:
    for _p in ("/opt/trn_rl_repo", os.path.expanduser("~/.axon_site/_ro/trn_rl_repo")):
        if os.path.isdir(_p) and _p not in sys.path:
            sys.path.insert(0, _p)
    import concourse.bass as bass

import concourse.mybir as mybir
import concourse.tile as tile
from concourse import bacc
from concourse.masks import make_identity
from contextlib import ExitStack

F32 = mybir.dt.float32
F32R = mybir.dt.float32r
BF16 = mybir.dt.bfloat16
I16 = mybir.dt.int16
EXP = mybir.ActivationFunctionType.Exp
ALU = mybir.AluOpType

P = 128
HID = 256
NHEAD = 4
DHEAD = 64
NQ = 1024  # queries per core
NK = 2048  # keys (full)
QBLK = 512
NQB = NQ // QBLK  # 2
NKT = NK // P  # 16
NPAIR = NKT // 2  # 8
NCT = HID // P  # 2

FE_MUL = float(np.log2(np.e)) * 128.0  # folded into Wq on host
ACT_SCALE = 1.0 / FE_MUL
AMQ_KEEP = 16250  # 127*128 minus centering delta 6
AMQ_KILL = 8192   # masked lanes -> bf16 2^-63 ~ 0

# engine per kt-pair: D = DVE fast-exp, A = Act exp (spread to avoid
# consecutive-A runs starving the 2-slot score-PSUM rotation)
PAIR_ENG = ["D", "A", "A", "D", "A", "A", "D", "A"]
ACT_KTS = [2, 3, 4, 5, 8, 9, 10, 11, 14, 15]
DVE_KTS = [0, 1, 6, 7, 12, 13]
ACT_POS = {kt: j for j, kt in enumerate(ACT_KTS)}
DVE_POS = {kt: j for j, kt in enumerate(DVE_KTS)}


def build() -> bass.Bass:
    nc = bacc.Bacc()
    xqT = nc.declare_dram_parameter("xqT", [HID, NQ], BF16, isOutput=False)
    xkT = nc.declare_dram_parameter("xkT", [HID, NK], BF16, isOutput=False)
    amf = nc.declare_dram_parameter("amf", [len(ACT_KTS) * P, NQ], BF16, isOutput=False)
    amq = nc.declare_dram_parameter("amq", [len(DVE_KTS) * P, NQ], I16, isOutput=False)
    wqT = nc.declare_dram_parameter("wqT", [HID, HID], BF16, isOutput=False)
    wkT = nc.declare_dram_parameter("wkT", [HID, HID], BF16, isOutput=False)
    wvT = nc.declare_dram_parameter("wvT", [HID, HID], BF16, isOutput=False)
    woT = nc.declare_dram_parameter("woT", [HID, HID], F32R, isOutput=False)
    out_d = nc.declare_dram_parameter("out", [NQ, HID], F32, isOutput=True)

    with tile.TileContext(nc) as tc, ExitStack() as ctx:
        const = ctx.enter_context(tc.tile_pool(name="const", bufs=1))
        big = ctx.enter_context(tc.tile_pool(name="big", bufs=1))
        ptp = ctx.enter_context(tc.tile_pool(name="ptp", bufs=1))
        amp = ctx.enter_context(tc.tile_pool(name="amp", bufs=1))
        wrk = ctx.enter_context(tc.tile_pool(name="wrk", bufs=2))
        wrk1 = ctx.enter_context(tc.tile_pool(name="wrk1", bufs=1))
        ps_st = ctx.enter_context(tc.tile_pool(name="ps_st", bufs=3, space="PSUM"))
        ps_a = ctx.enter_context(tc.tile_pool(name="ps_a", bufs=1, space="PSUM"))
        ps_o = ctx.enter_context(tc.tile_pool(name="ps_o", bufs=1, space="PSUM"))

        # --- DMA loads (the DMA device serializes; K side first) ---
        def load2(name, src, width, dt=F32R):
            ts = []
            for t in range(2):
                tl = const.tile([P, width], dt, tag=f"{name}{t}", name=f"{name}{t}")
                nc.sync.dma_start(out=tl, in_=src[t * P : (t + 1) * P, :])
                ts.append(tl)
            return ts

        wk_sb = load2("wk", wkT, HID, dt=BF16)
        xk_sb = []
        for t in range(2):
            tl = big.tile([P, NK], BF16, tag=f"xk{t}", name=f"xk{t}")
            nc.sync.dma_start(out=tl, in_=xkT[t * P : (t + 1) * P, :])
            xk_sb.append(tl)
        wq_sb = load2("wq", wqT, HID, dt=BF16)
        wv_sb = load2("wv", wvT, HID, dt=BF16)
        xq_sb = []
        for t in range(2):
            tl = big.tile([P, NQ], BF16, tag=f"xq{t}", name=f"xq{t}")
            nc.sync.dma_start(out=tl, in_=xqT[t * P : (t + 1) * P, :])
            xq_sb.append(tl)

        amf_r = amf.rearrange("(t p) q -> p t q", p=P)
        amq_r = amq.rearrange("(t p) q -> p t q", p=P)
        am_sb = {}   # (qb, act_pos_start) -> tile [P, w, QBLK] bf16
        amq_sb = {}  # (qb, dve_pos_start) -> tile [P, w, QBLK] i16

        def load_mask_chunks(qb):
            # one chunk per consumption unit, in pair order, so the DMA
            # arrivals pace the pipeline
            qsl = slice(qb * QBLK, (qb + 1) * QBLK)
            a_pos = 0
            d_pos = 0
            for pj in range(NPAIR):
                eng = PAIR_ENG[pj]
                if eng in ("D", "S"):
                    w = 1 if eng == "S" else 2
                    tq = amp.tile(
                        [P, w, QBLK], I16, tag=f"amq_{d_pos}", name=f"amq_{d_pos}"
                    )
                    nc.sync.dma_start(out=tq, in_=amq_r[:, d_pos : d_pos + w, qsl])
                    amq_sb[(qb, d_pos)] = tq
                    d_pos += w
                if eng in ("A", "S"):
                    w = 1 if eng == "S" else 2
                    tl = amp.tile(
                        [P, w, QBLK], BF16, tag=f"am_{a_pos}", name=f"am_{a_pos}"
                    )
                    nc.sync.dma_start(out=tl, in_=amf_r[:, a_pos : a_pos + w, qsl])
                    am_sb[(qb, a_pos)] = tl
                    a_pos += w

        load_mask_chunks(0)
        wo_sb = load2("wo", woT, HID)

        ident_f = const.tile([P, P], F32, tag="ident_f", name="ident_f")
        make_identity(nc, ident_f)
        ident = const.tile([P, P], F32R, tag="ident", name="ident")
        nc.vector.tensor_copy(ident, ident_f)

        # --- projections (copies alternate Act/DVE; prologue work) ---
        cp_i = 0

        def copy_eng(out, in_):
            nonlocal cp_i
            cp_i += 1
            if cp_i % 2 == 0:
                nc.scalar.copy(out, in_)
            else:
                nc.vector.tensor_copy(out, in_)

        kt_sb = [big.tile([P, NK], F32R, tag=f"kt{t}", name=f"kt{t}") for t in range(2)]
        for t in range(2):
            for nb in range(NK // (2 * QBLK)):
                ps = ps_st.tile([P, 2 * QBLK], F32, tag="st", name="st")
                for half in range(2):
                    for ct in range(NCT):
                        nc.tensor.matmul(
                            ps[:, half * QBLK : (half + 1) * QBLK],
                            lhsT=wk_sb[ct][:, t * P : (t + 1) * P],
                            rhs=xk_sb[ct][
                                :, (2 * nb + half) * QBLK : (2 * nb + half + 1) * QBLK
                            ],
                            start=(ct == 0),
                            stop=(ct == NCT - 1),
                        )
                copy_eng(kt_sb[t][:, 2 * nb * QBLK : (2 * nb + 2) * QBLK], ps)

        qt_sb = [big.tile([P, NQ], F32R, tag=f"qt{t}", name=f"qt{t}") for t in range(2)]
        for t in range(2):
            ps = ps_st.tile([P, 2 * QBLK], F32, tag="st", name="st")
            for half in range(2):
                for ct in range(NCT):
                    nc.tensor.matmul(
                        ps[:, half * QBLK : (half + 1) * QBLK],
                        lhsT=wq_sb[ct][:, t * P : (t + 1) * P],
                        rhs=xq_sb[ct][:, half * QBLK : (half + 1) * QBLK],
                        start=(ct == 0),
                        stop=(ct == NCT - 1),
                    )
            copy_eng(qt_sb[t], ps)

        # V''[ktok, h, 0:64] = V rows (bf16); V''[ktok, h, 64] = 1.0
        vpp = [None] * NKT
        for kt2 in range(NKT // 2):
            ps = ps_st.tile([P, 2 * QBLK], F32, tag="st", name="st")
            for half in range(2):
                kt = 2 * kt2 + half
                for ct in range(NCT):
                    nc.tensor.matmul(
                        ps[:, half * QBLK : half * QBLK + HID],
                        lhsT=xk_sb[ct][:, kt * P : (kt + 1) * P],
                        rhs=wv_sb[ct],
                        start=(ct == 0),
                        stop=(ct == NCT - 1),
                    )
            for half in range(2):
                kt = 2 * kt2 + half
                tl = big.tile([P, NHEAD, DHEAD + 1], BF16, tag=f"v{kt}", name=f"v{kt}")
                copy_eng(
                    tl[:, :, 0:DHEAD],
                    ps[:, half * QBLK : half * QBLK + HID].rearrange(
                        "p (h d) -> p h d", h=NHEAD
                    ),
                )
                nc.gpsimd.memset(tl[:, :, DHEAD : DHEAD + 1], 1.0)
                vpp[kt] = tl

        # --- attention ---
        def emit_head(qb, h, pts):
            qsl = slice(qb * QBLK, (qb + 1) * QBLK)
            t, po = h // 2, (h % 2) * DHEAD
            for pj in range(NPAIR):
                ps = ps_st.tile([P, 2 * QBLK], F32, tag="st", name="st")
                for half in range(2):
                    kt = 2 * pj + half
                    nc.tensor.matmul(
                        ps[:, half * QBLK : (half + 1) * QBLK],
                        lhsT=kt_sb[t][po : po + DHEAD, kt * P : (kt + 1) * P],
                        rhs=qt_sb[t][po : po + DHEAD, qsl],
                        start=True,
                        stop=True,
                    )
                eng = PAIR_ENG[pj]
                tg = f"pt{h}_{pj}" if h > 2 else f"pt{h}{qb % 2}_{pj}"
                if eng == "A":
                    pt = ptp.tile([P, 2 * QBLK], BF16, tag=tg, name=tg)
                    nc.scalar.activation(pt, ps, EXP, scale=ACT_SCALE)
                    j = ACT_POS[2 * pj]
                    nc.gpsimd.tensor_mul(
                        pt.rearrange("p (two q) -> p two q", two=2),
                        pt.rearrange("p (two q) -> p two q", two=2),
                        am_sb[(qb, j)],
                    )
                    for half in range(2):
                        pts[(h, 2 * pj + half)] = pt[:, half * QBLK : (half + 1) * QBLK]
                elif eng == "D":
                    ie = ptp.tile([P, 2 * QBLK], I16, tag=tg, name=tg)
                    j = DVE_POS[2 * pj]
                    nc.vector.tensor_add(
                        ie.rearrange("p (two q) -> p two q", two=2),
                        ps.rearrange("p (two q) -> p two q", two=2),
                        amq_sb[(qb, j)],
                    )
                    pv = ie.bitcast(BF16)
                    for half in range(2):
                        pts[(h, 2 * pj + half)] = pv[:, half * QBLK : (half + 1) * QBLK]
                else:  # split pair: kt14 -> DVE, kt15 -> Act
                    ie = ptp.tile([P, QBLK], I16, tag=f"pti{h}_{pj}", name=f"pti{h}_{pj}")
                    jd = DVE_POS[2 * pj]
                    nc.vector.tensor_add(
                        ie, ps[:, 0:QBLK], amq_sb[(qb, jd)][:, 0, :]
                    )
                    pts[(h, 2 * pj)] = ie.bitcast(BF16)
                    pt = ptp.tile([P, QBLK], BF16, tag=f"pta{h}_{pj}", name=f"pta{h}_{pj}")
                    nc.scalar.activation(pt, ps[:, QBLK : 2 * QBLK], EXP, scale=ACT_SCALE)
                    ja = ACT_POS[2 * pj + 1]
                    nc.gpsimd.tensor_mul(pt, pt, am_sb[(qb, ja)][:, 0, :])
                    pts[(h, 2 * pj + 1)] = pt

        deferred = []

        def run_deferred():
            if deferred:
                deferred.pop(0)()

        def process_qb(qb):
            if qb > 0:
                load_mask_chunks(qb)
            pts = {}
            ps_avs = {}

            def get_ps_av(qt):
                if qt not in ps_avs:
                    if qt < 2:
                        ps_avs[qt] = ps_a.tile(
                            [P, NHEAD * (DHEAD + 1)], F32, tag="a0",
                            name="a0", padded_shape=[P, QBLK],
                        )
                    else:
                        # tail-only: borrow an idle score-pool slot
                        big_t = ps_st.tile([P, 2 * QBLK], F32, tag="st", name="st_a")
                        ps_avs[qt] = big_t[:, 0 : NHEAD * (DHEAD + 1)]
                return ps_avs[qt]

            def emit_chain(qt, h):
                ps_av = get_ps_av(qt)
                for kt in range(NKT):
                    nc.tensor.matmul(
                        ps_av[:, h * 65 : (h + 1) * 65],
                        lhsT=pts[(h, kt)][:, qt * P : (qt + 1) * P],
                        rhs=vpp[kt][:, h, :],
                        start=(kt == 0),
                        stop=(kt == NKT - 1),
                    )

            def emit_norm(qt):
                ps_av = ps_avs.pop(qt)
                rec = wrk.tile([P, NHEAD], F32, tag=f"rec{qt % 2}", name=f"rec{qt % 2}")
                nc.vector.reciprocal(rec, ps_av[:, DHEAD : NHEAD * 65 : 65])
                an = wrk1.tile([P, HID], F32R, tag=f"an{qt % 2}", name=f"an{qt % 2}")
                for h in range(NHEAD):
                    nc.vector.tensor_scalar_mul(
                        an[:, h * DHEAD : (h + 1) * DHEAD],
                        ps_av[:, h * 65 : h * 65 + DHEAD],
                        rec[:, h : h + 1],
                    )
                return an

            def emit_o(qt, an):
                o_ps = ps_o.tile([P, QBLK], F32, tag="o", name="o_ps")
                atts = []
                for ct in range(NCT):
                    tp = o_ps[:, HID + ct * P : HID + (ct + 1) * P].bitcast(F32R)
                    nc.tensor.transpose(tp, an[:, ct * P : (ct + 1) * P], ident)
                    att = wrk.tile([P, P], F32R, tag=f"att{ct}", name=f"att{ct}")
                    nc.vector.tensor_copy(att, tp)
                    atts.append(att)
                for ct in range(NCT):
                    nc.tensor.matmul(
                        o_ps[:, 0:HID],
                        lhsT=atts[ct],
                        rhs=wo_sb[ct],
                        start=(ct == 0),
                        stop=(ct == NCT - 1),
                    )
                ob = wrk1.tile([P, HID], F32, tag=f"ob{qt % 2}", name=f"ob{qt % 2}")
                nc.vector.tensor_copy(ob, o_ps[:, 0:HID])
                q0 = qb * QBLK + qt * P
                nc.sync.dma_start(out=out_d[q0 : q0 + P, :], in_=ob)

            for h in range(NHEAD):
                emit_head(qb, h, pts)
                if h in (1, 2):
                    # previous qb's deferred tail chains/outputs overlap
                    # this qb's first head phases (two slices to spread the
                    # extra PE load)
                    run_deferred()
                if h > 0:
                    emit_chain(0, h - 1)
            emit_chain(0, NHEAD - 1)
            an0 = emit_norm(0)

            last = qb == NQB - 1

            def tail1(emit_chain=emit_chain, emit_norm=emit_norm,
                      emit_o=emit_o, an0=an0):
                emit_o(0, an0)
                for h in range(NHEAD):
                    emit_chain(1, h)
                an1 = emit_norm(1)
                emit_o(1, an1)

            def tail2(emit_chain=emit_chain, emit_norm=emit_norm,
                      emit_o=emit_o):
                for h in range(NHEAD):
                    emit_chain(2, h)
                an2 = emit_norm(2)
                for h in range(NHEAD):
                    emit_chain(3, h)
                emit_o(2, an2)
                an3 = emit_norm(3)
                emit_o(3, an3)

            deferred.append(tail1)
            deferred.append(tail2)

        for qb in range(NQB):
            process_qb(qb)
        run_deferred()
        run_deferred()
    nc.compile()
    return nc


_NC_CACHE = {}
_last_in_maps = None


def _get_nc(with_attn_mask: bool = False) -> bass.Bass:
    key = "v5"
    if key not in _NC_CACHE:
        _NC_CACHE[key] = build()
    return _NC_CACHE[key]


def kernel(q_hidden_states, k_hidden_states, attention_mask, align_mask, Wq, Wk, Wv, Wo):
    from concourse.bass_utils import run_bass_kernel_spmd

    q_hidden_states = np.asarray(q_hidden_states, np.float32)
    k_hidden_states = np.asarray(k_hidden_states, np.float32)
    attention_mask = np.asarray(attention_mask, np.float32)
    align_mask = np.asarray(align_mask)
    B, Q, _ = q_hidden_states.shape
    qh_len = Q // 2  # 1024

    nc = _get_nc()

    # scores arrive in PSUM pre-scaled by log2e*128 (folded into Wq here)
    wq = np.ascontiguousarray(
        (np.asarray(Wq, np.float32).T * np.float32(FE_MUL / 8.0)).astype(ml_dtypes.bfloat16)
    )
    wk = np.ascontiguousarray(np.asarray(Wk, np.float32).T.astype(ml_dtypes.bfloat16))
    wv = np.ascontiguousarray(np.asarray(Wv, np.float32).T.astype(ml_dtypes.bfloat16))
    wo = np.ascontiguousarray(np.asarray(Wo, np.float32).T)

    if np.any(attention_mask):
        raise NotImplementedError("nonzero additive attention_mask not supported")

    in_maps = []
    for core in range(8):
        b, qh = divmod(core, 2)
        qsl = slice(qh * qh_len, (qh + 1) * qh_len)
        am = align_mask[b, :, qsl]
        am_act = np.concatenate([am[kt * 128 : (kt + 1) * 128] for kt in ACT_KTS])
        am_dve = np.concatenate([am[kt * 128 : (kt + 1) * 128] for kt in DVE_KTS])
        m = {
            "xqT": np.ascontiguousarray(q_hidden_states[b, qsl].T.astype(ml_dtypes.bfloat16)),
            "xkT": np.ascontiguousarray(k_hidden_states[b].T.astype(ml_dtypes.bfloat16)),
            "amf": np.ascontiguousarray(am_act.astype(ml_dtypes.bfloat16)),
            "amq": np.ascontiguousarray(
                np.where(am_dve != 0, AMQ_KEEP, AMQ_KILL).astype(np.int16)
            ),
            "wqT": wq,
            "wkT": wk,
            "wvT": wv,
            "woT": wo,
        }
        in_maps.append(m)

    global _last_in_maps
    _last_in_maps = in_maps
    res = run_bass_kernel_spmd(nc, in_maps, list(range(8))).results
    out = np.empty((B, Q, HID), np.float32)
    for core in range(8):
        b, qh = divmod(core, 2)
        out[b, qh * qh_len : (qh + 1) * qh_len] = res[core]["out"]
    return out
